# revision 2
# baseline (speedup 1.0000x reference)
"""ConstituencyTreeLSTM Trainium2 kernel.

Strategy:
  - Data-parallel over the B=256 batch across 8 NeuronCores (32 rows/core).
  - The tree is a complete heap (node i has children 2i+1, 2i+2), so the
    sequential scan is reorganized into level-parallel phases:
      leaves (nodes 128..255) -> node 127 -> level 6 (63..126) -> ... -> root.
  - Everything on-device lives in a "feature-on-partitions, (node,batch) rows
    on free axis" layout, so matmul outputs (PSUM, [out_dim, rows]) are already
    in the layout needed to feed the next level's matmul. No transposes.
  - One fused bf16 weight matrix W_big [1536, 2560]:
      rows:  0:512 x | 512:1024 hL | 1024:1536 hR
      cols:  0:1536 iou | 1536:2048 fL-pre | 2048:2560 fR-pre
    Zero blocks (hR->fL, hL->fR) are skipped; only the 208 used 128x128
    blocks are stored (packed).
  - h of every level lives in SBUF level tiles; parents read children h via
    stride-2 node slices directly (no DRAM roundtrip on the critical path).
  - c goes through DRAM (CL/CR, parity-split by parent index) - it is only
    needed by the cheap elementwise stage, late in each chunk.
  - Per-node-type biases (2-child / leaf / 1-child) folded host-side and
    applied inside the PSUM-evacuating activation (sigmoid/tanh).
"""

import sys

sys.path.insert(0, "/opt/trn_rl_repo")

import numpy as np
import ml_dtypes

import concourse.bass as bass  # noqa: F401
import concourse.mybir as mybir
import concourse.tile as tile
from concourse import bacc
from concourse.bass_utils import run_bass_kernel_spmd

BF16 = ml_dtypes.bfloat16
NCORES = 8
B, N, D = 256, 256, 512
BC = B // NCORES  # batch rows per core
KT_X, KT_HL, KT_HR = range(0, 4), range(4, 8), range(8, 12)
NJ = 20  # output j-tiles: 12 iou + 4 fL + 4 fR

_compiled = {}


def _used_kts(j, has_l=True, has_r=True):
    if j < 12:
        kts = list(KT_X) + (list(KT_HL) if has_l else []) + (list(KT_HR) if has_r else [])
    elif j < 16:
        kts = list(KT_X) + list(KT_HL)
    else:
        kts = list(KT_X) + list(KT_HR)
    return kts


# packed weight-block index: only (kt, j) pairs with nonzero weight blocks
W_BLOCKS = [(kt, j) for j in range(NJ) for kt in _used_kts(j)]
W_IDX = {p: i for i, p in enumerate(W_BLOCKS)}
NW = len(W_BLOCKS)  # 208


def _build_bass(reps=1, kts_limit=None, skip_ew=False):
    nc = bacc.Bacc("TRN2", target_bir_lowering=False, debug=False, num_devices=NCORES)

    f32 = mybir.dt.float32
    bf16 = mybir.dt.bfloat16

    xt = nc.dram_tensor("xt", [N, D, BC], bf16, kind="ExternalInput")
    w = nc.dram_tensor("w", [NW, 128, 128], bf16, kind="ExternalInput")
    b2_d = nc.dram_tensor("b2", [128, NJ], f32, kind="ExternalInput")
    bleaf_d = nc.dram_tensor("bleaf", [128, NJ], f32, kind="ExternalInput")
    b1_d = nc.dram_tensor("b1", [128, NJ], f32, kind="ExternalInput")

    # children c keyed by parent index t: CL[t] = c(2t+1), CR[t] = c(2t+2)
    CL = nc.dram_tensor("CLbuf", [128, D, BC], bf16)
    CR = nc.dram_tensor("CRbuf", [128, D, BC], bf16)

    c0t = nc.dram_tensor("c0t", [D, BC], f32, kind="ExternalOutput")
    h0t = nc.dram_tensor("h0t", [D, BC], f32, kind="ExternalOutput")

    # all views are [partition, node, ktile, batch]; (node, ktile) merge on DMA
    xt_r = xt.ap().rearrange("n (kt p) b -> p n kt b", p=128)
    CL_r = CL.ap().rearrange("t (kt p) b -> p t kt b", p=128)
    CR_r = CR.ap().rearrange("t (kt p) b -> p t kt b", p=128)
    c0t_r = c0t.ap().rearrange("(kt p) b -> p kt b", p=128)
    h0t_r = h0t.ap().rearrange("(kt p) b -> p kt b", p=128)

    with tile.TileContext(nc) as tc:
        import contextlib

        ctx = contextlib.ExitStack()
        with ctx:
            wpool = ctx.enter_context(tc.tile_pool(name="wpool", bufs=1))
            hpool = ctx.enter_context(tc.tile_pool(name="hpool", bufs=1))
            inpool = ctx.enter_context(tc.tile_pool(name="inpool", bufs=2))
            gpool = ctx.enter_context(tc.tile_pool(name="gpool", bufs=2))
            epool = ctx.enter_context(tc.tile_pool(name="epool", bufs=2))
            pspool = ctx.enter_context(tc.tile_pool(name="ps", bufs=8, space="PSUM"))

            w_sb = wpool.tile([128, NW, 128], bf16)
            nc.sync.dma_start(out=w_sb[:], in_=w.ap().rearrange("blk p c -> p blk c"))
            b2_sb = wpool.tile([128, NJ], f32, name="b2sb")
            bleaf_sb = wpool.tile([128, NJ], f32, name="bleafsb")
            b1_sb = wpool.tile([128, NJ], f32, name="b1sb")
            nc.sync.dma_start(out=b2_sb[:], in_=b2_d.ap()[:])
            nc.sync.dma_start(out=bleaf_sb[:], in_=bleaf_d.ap()[:])
            nc.sync.dma_start(out=b1_sb[:], in_=b1_d.ap()[:])

            def process(
                nodes,
                has_l,
                has_r,
                bias_sb,
                child_h,  # (tile, base_node) or None
                out_h,  # (tile, base_node) or None (root)
                child_c=None,  # (tile, base_node) -> read children c from SBUF
                out_c=None,  # (tile, base_node) -> write c to SBUF, skip CL/CR
            ):
                """Compute (c, h) for `nodes` (a range), all at the same depth."""
                to_out = out_h is None
                for a in range(nodes.start, nodes.stop, 16):
                    b_ = min(a + 16, nodes.stop)
                    k = b_ - a  # nodes in this chunk
                    dt_g = f32 if to_out else bf16

                    xt_t = inpool.tile([128, k, 4, BC], bf16, name="xt_t")
                    nc.sync.dma_start(out=xt_t[:], in_=xt_r[:, a:b_, :, :])
                    if child_c is None:
                        if has_l:
                            cl_t = inpool.tile([128, k, 4, BC], bf16, name="cl_t")
                            nc.sync.dma_start(out=cl_t[:], in_=CL_r[:, a:b_, :, :])
                        if has_r:
                            cr_t = inpool.tile([128, k, 4, BC], bf16, name="cr_t")
                            nc.sync.dma_start(out=cr_t[:], in_=CR_r[:, a:b_, :, :])
                    if child_h is not None:
                        ch_t, ch_base = child_h
                        sl0 = 2 * a + 1 - ch_base

                        def child_slice(kt, off):
                            s0 = sl0 + off
                            if k == 1:
                                return ch_t[:, s0 : s0 + 1, kt, :]
                            return ch_t[:, s0 : s0 + 2 * k - 1 : 2, kt, :]

                    if child_c is not None:
                        cc_t, cc_base = child_c
                        cs0 = 2 * a + 1 - cc_base
                        if k == 1:
                            cl_t = cc_t[:, cs0 : cs0 + 1, :, :]
                            cr_t = cc_t[:, cs0 + 1 : cs0 + 2, :, :]
                        else:
                            cl_t = cc_t[:, cs0 : cs0 + 2 * k - 1 : 2, :, :]
                            cr_t = cc_t[:, cs0 + 1 : cs0 + 2 * k : 2, :, :]

                    g_i = gpool.tile([128, k, 4, BC], dt_g, name="g_i")
                    g_o = gpool.tile([128, k, 4, BC], dt_g, name="g_o")
                    g_u = gpool.tile([128, k, 4, BC], dt_g, name="g_u")
                    if has_l:
                        g_fl = gpool.tile([128, k, 4, BC], dt_g, name="g_fl", bufs=1)
                    if has_r:
                        g_fr = gpool.tile([128, k, 4, BC], dt_g, name="g_fr", bufs=1)

                    js = list(range(12))
                    if has_l:
                        js += list(range(12, 16))
                    if has_r:
                        js += list(range(16, 20))

                    for j in js:
                        kts = _used_kts(j, has_l, has_r)
                        if kts_limit:
                            kts = kts[:kts_limit]

                        ps = pspool.tile([128, k, BC], f32, name="ps")
                        for i, kt in enumerate(kts):
                            if kt < 4:
                                rhs = xt_t[:, :, kt, :]
                            elif kt < 8:
                                rhs = child_slice(kt - 4, 0)
                            else:
                                rhs = child_slice(kt - 8, 1)
                            nc.tensor.matmul(
                                ps[:],
                                w_sb[:, W_IDX[(kt, j)], :],
                                rhs,
                                start=(i == 0),
                                stop=(i == len(kts) - 1),
                            )
                        func = (
                            mybir.ActivationFunctionType.Tanh
                            if 8 <= j < 12
                            else mybir.ActivationFunctionType.Sigmoid
                        )
                        if j < 4:
                            dst = g_i[:, :, j, :]
                        elif j < 8:
                            dst = g_o[:, :, j - 4, :]
                        elif j < 12:
                            dst = g_u[:, :, j - 8, :]
                        elif j < 16:
                            dst = g_fl[:, :, j - 12, :]
                        else:
                            dst = g_fr[:, :, j - 16, :]
                        nc.scalar.activation(
                            out=dst,
                            in_=ps[:],
                            func=func,
                            bias=bias_sb[:, j : j + 1],
                            scale=1.0,
                        )

                    if skip_ew:
                        continue

                    # c = i*u (+ fl*cl) (+ fr*cr);  h = o * tanh(c)
                    if out_c is not None:
                        oc_t, oc_base = out_c
                        c_t = oc_t[:, a - oc_base : b_ - oc_base, :, :]
                    else:
                        c_t = epool.tile([128, k, 4, BC], dt_g, name="c_t")
                    nc.vector.tensor_mul(c_t[:], g_i[:], g_u[:])
                    if has_l:
                        m2 = epool.tile([128, k, 4, BC], dt_g, name="mt")
                        nc.vector.tensor_mul(m2[:], g_fl[:], cl_t[:])
                        nc.vector.tensor_add(c_t[:], c_t[:], m2[:])
                    if has_r:
                        m3 = epool.tile([128, k, 4, BC], dt_g, name="mt")
                        nc.vector.tensor_mul(m3[:], g_fr[:], cr_t[:])
                        nc.vector.tensor_add(c_t[:], c_t[:], m3[:])
                    tc_t = epool.tile([128, k, 4, BC], dt_g, name="tc_t")
                    nc.scalar.activation(
                        out=tc_t[:], in_=c_t[:], func=mybir.ActivationFunctionType.Tanh
                    )

                    if to_out:
                        h_t = epool.tile([128, k, 4, BC], dt_g, name="h_t")
                        nc.vector.tensor_mul(h_t[:], g_o[:], tc_t[:])
                        nc.sync.dma_start(out=c0t_r[:], in_=c_t[:, 0, :, :])
                        nc.sync.dma_start(out=h0t_r[:], in_=h_t[:, 0, :, :])
                    else:
                        oh_t, oh_base = out_h
                        nc.vector.tensor_mul(
                            oh_t[:, a - oh_base : b_ - oh_base, :, :], g_o[:], tc_t[:]
                        )
                        if out_c is not None:
                            continue  # c already written to its SBUF level tile
                        # c of node t -> CL[(t-1)//2] if t odd else CR[t//2 - 1]
                        odd0 = 0 if a % 2 == 1 else 1
                        even0 = 1 - odd0
                        odds = range(a + odd0, b_, 2)
                        evens = range(a + even0, b_, 2)
                        for kt in range(4):
                            if len(odds):
                                lo = (odds[0] - 1) // 2
                                nc.sync.dma_start(
                                    out=CL_r[:, lo : lo + len(odds), kt, :],
                                    in_=c_t[:, odd0::2, kt, :],
                                )
                            if len(evens):
                                ro = evens[0] // 2 - 1
                                nc.sync.dma_start(
                                    out=CR_r[:, ro : ro + len(evens), kt, :],
                                    in_=c_t[:, even0::2, kt, :],
                                )

            # c stays in SBUF for the small tail levels (outputs of L4..L1);
            # their parent phases then skip the CL/CR DRAM roundtrip entirely.
            C_SBUF_LVLS = (4, 3, 2, 1)

            for _rep in range(reps):
                # per-level h tiles (SBUF-resident)
                leafc_h = hpool.tile([128, 129, 4, BC], bf16, name="h_leafc")
                lvl_h = {7: (leafc_h, 127)}
                lvl_c = {}
                for lvl in range(6, 0, -1):
                    t = hpool.tile([128, 2**lvl, 4, BC], bf16, name=f"h_{lvl}")
                    lvl_h[lvl] = (t, 2**lvl - 1)
                for lvl in C_SBUF_LVLS:
                    t = hpool.tile([128, 2**lvl, 4, BC], bf16, name=f"c_{lvl}")
                    lvl_c[lvl] = (t, 2**lvl - 1)

                # leaves: nodes 128..255 (no children)
                process(range(128, 256), False, False, bleaf_sb, None, lvl_h[7])
                # node 127: left child only (node 255, leafc slot 128)
                process(range(127, 128), True, False, b1_sb, lvl_h[7], lvl_h[7])
                # levels 6..1: two children each
                for lvl in range(6, 0, -1):
                    process(
                        range(2**lvl - 1, 2 ** (lvl + 1) - 1),
                        True,
                        True,
                        b2_sb,
                        lvl_h[lvl + 1] if lvl < 6 else lvl_h[7],
                        lvl_h[lvl],
                        child_c=lvl_c.get(lvl + 1),
                        out_c=lvl_c.get(lvl),
                    )
                # root
                process(range(0, 1), True, True, b2_sb, lvl_h[1], None, child_c=lvl_c.get(1))

    nc.compile()
    return nc


def _expected_tree():
    left = np.array([2 * i + 1 if 2 * i + 1 < N else 0 for i in range(N)], np.int32)
    right = np.array([2 * i + 2 if 2 * i + 2 < N else 0 for i in range(N)], np.int32)
    nch = np.array(
        [int(2 * i + 1 < N) + int(2 * i + 2 < N) for i in range(N)], np.int32
    )
    return left, right, nch


def pack_w(W_ioux, W_fx, W_iouhL, W_fhL, W_iouhR, W_fhR):
    w_big = np.zeros((1536, 2560), np.float32)
    w_big[0:512, 0:1536] = np.asarray(W_ioux, np.float32).T
    w_big[0:512, 1536:2048] = np.asarray(W_fx, np.float32).T
    w_big[0:512, 2048:2560] = np.asarray(W_fx, np.float32).T
    w_big[512:1024, 0:1536] = np.asarray(W_iouhL, np.float32).T
    w_big[512:1024, 1536:2048] = np.asarray(W_fhL, np.float32).T
    w_big[1024:1536, 0:1536] = np.asarray(W_iouhR, np.float32).T
    w_big[1024:1536, 2048:2560] = np.asarray(W_fhR, np.float32).T
    w_np = np.empty((NW, 128, 128), np.float32)
    for i, (kt, j) in enumerate(W_BLOCKS):
        w_np[i] = w_big[kt * 128 : (kt + 1) * 128, j * 128 : (j + 1) * 128]
    return np.ascontiguousarray(w_np).astype(BF16)


def pack_biases(b_ioux, b_iouh, b_iouhL, b_iouhR, b_fx, b_fhL, b_fhR):
    def pack(vec):
        return np.ascontiguousarray(np.asarray(vec, np.float32).reshape(NJ, 128).T)

    z = np.zeros(512, np.float32)
    b2 = pack(np.concatenate([b_ioux + b_iouhL + b_iouhR, b_fx + b_fhL, b_fx + b_fhR]))
    bleaf = pack(np.concatenate([b_ioux + b_iouh, z, z]))
    b1 = pack(np.concatenate([b_ioux + b_iouhL, b_fx + b_fhL, z]))
    return b2, bleaf, b1


def prepare_in_maps(np_inputs):
    i = np_inputs
    inputs = np.asarray(i["inputs"], np.float32)
    w_np = pack_w(
        i["W_ioux"], i["W_fx"], i["W_iouhL"], i["W_fhL"], i["W_iouhR"], i["W_fhR"]
    )
    b_args = [
        np.asarray(i[k], np.float32)
        for k in ("b_ioux", "b_iouh", "b_iouhL", "b_iouhR", "b_fx", "b_fhL", "b_fhR")
    ]
    b2, bleaf, b1 = pack_biases(*b_args)

    in_maps = []
    for c in range(NCORES):
        xc = inputs[c * BC : (c + 1) * BC]  # [BC, N, D]
        xt_c = np.ascontiguousarray(xc.transpose(1, 2, 0)).astype(BF16)  # [N, D, BC]
        in_maps.append({"xt": xt_c, "w": w_np, "b2": b2, "bleaf": bleaf, "b1": b1})
    return in_maps


def kernel(
    inputs,
    W_ioux, b_ioux, W_iouh, b_iouh, W_iouhL, b_iouhL, W_iouhR, b_iouhR,
    W_fx, b_fx, W_fh, b_fh, W_fhL, b_fhL, W_fhR, b_fhR,
    left_idx, right_idx, num_children,
):
    el, er, en = _expected_tree()
    assert np.array_equal(np.asarray(left_idx), el), "unexpected tree structure"
    assert np.array_equal(np.asarray(right_idx), er), "unexpected tree structure"
    assert np.array_equal(np.asarray(num_children), en), "unexpected tree structure"

    in_maps = prepare_in_maps(
        dict(
            inputs=inputs,
            W_ioux=W_ioux, W_fx=W_fx, W_iouhL=W_iouhL, W_fhL=W_fhL,
            W_iouhR=W_iouhR, W_fhR=W_fhR,
            b_ioux=b_ioux, b_iouh=b_iouh, b_iouhL=b_iouhL, b_iouhR=b_iouhR,
            b_fx=b_fx, b_fhL=b_fhL, b_fhR=b_fhR,
        )
    )

    if "nc" not in _compiled:
        _compiled["nc"] = _build_bass()
    nc = _compiled["nc"]

    res = run_bass_kernel_spmd(
        nc, in_maps, core_ids=list(range(NCORES)), trace=bool(_compiled.get("trace"))
    )
    _compiled["last_res"] = res

    c_full = np.empty((B, D), np.float32)
    h_full = np.empty((B, D), np.float32)
    for c in range(NCORES):
        c_full[c * BC : (c + 1) * BC] = res.results[c]["c0t"].T
        h_full[c * BC : (c + 1) * BC] = res.results[c]["h0t"].T
    return c_full, h_full



# revision 16
# speedup vs baseline: 1.5387x; 1.5387x over previous
"""ConstituencyTreeLSTM Trainium2 kernel (fp8 DoubleRow edition).

Strategy:
  - Data-parallel over the B=256 batch across 8 NeuronCores (32 rows/core).
  - Complete-heap tree -> level-parallel phases:
      leaves (128..255) -> node 127 -> L6 (63..126) -> ... -> L1 -> root.
  - Feature-on-partitions layout; matmul PSUM outputs feed the next level
    without transposes. All SBUF tiles are kt-major: [128, kt, node, batch].
  - Big levels (leaves, 127, L6, L5, L4) run matmuls in fp8e4 with
    MatmulPerfMode.DoubleRow: 2 k-tiles contracted per instruction at
    0.5 cycles/row -> 4x bf16 PE throughput, half the instructions.
    Weights are scaled x64 host-side (keeps uniform(+-1/sqrt(512)) weights
    in the fp8 normal range); the 1/64 is folded into the PSUM-evacuating
    activation's scale. x and h are quantized to fp8 unscaled.
  - Small levels (L3..root, 15 nodes) run in bf16: their PE time is
    negligible and this restores most of the accuracy (sim: 1.3e-2 vs
    3.7e-2 all-fp8, threshold 2e-2).
  - h lives in SBUF per level, parity-split by parent (left-children tile /
    right-children tile) so DoubleRow rhs slices stay dense: the stride-2
    child gather becomes a contiguous slice indexed by parent.
  - c of big levels goes through DRAM (CL/CR, parity-split by parent);
    c of L4..L1 stays in SBUF.
  - Per-node-type biases (2-child / leaf / 1-child) folded host-side,
    applied inside the PSUM-evacuating activation (sigmoid/tanh).
  - Elementwise c/h stage on DVE (2x bf16) with one mul offloaded to Pool.
"""

import sys

sys.path.insert(0, "/opt/trn_rl_repo")

import numpy as np
import ml_dtypes

import concourse.bass as bass  # noqa: F401
import concourse.mybir as mybir
import concourse.tile as tile
from concourse import bacc
from concourse.bass_utils import run_bass_kernel_spmd

BF16 = ml_dtypes.bfloat16
FP8 = ml_dtypes.float8_e4m3
NCORES = 8
B, N, D = 256, 256, 512
BC = B // NCORES  # batch rows per core
NJ = 20  # output j-tiles: 12 iou + 4 fL + 4 fR
WSCALE = 64.0  # fp8 weight scale; inverse folded into activation scale

N_TAIL = 15  # nodes 0..14 (L3..root) run in bf16

_compiled = {}


def _used_kts(j, has_l=True, has_r=True):
    if j < 12:
        kts = list(range(0, 4)) + (list(range(4, 8)) if has_l else []) + (
            list(range(8, 12)) if has_r else []
        )
    elif j < 16:
        kts = list(range(0, 4)) + list(range(4, 8))
    else:
        kts = list(range(0, 4)) + list(range(8, 12))
    return kts


# packed weight-block index: only (kt, j) pairs with nonzero weight blocks.
# The x-blocks of j16..19 (fR's W_fx) duplicate j12..15's (fL's W_fx) and are
# not stored; widx() remaps them.
W_BLOCKS = [
    (kt, j) for j in range(NJ) for kt in _used_kts(j) if not (j >= 16 and kt < 4)
]
W_IDX = {p: i for i, p in enumerate(W_BLOCKS)}
NW = len(W_BLOCKS)  # 192


def widx(kt, j):
    if j >= 16 and kt < 4:
        j = j - 4
    return W_IDX[(kt, j)]


def _build_bass(reps=1):
    nc = bacc.Bacc("TRN2", target_bir_lowering=False, debug=False, num_devices=NCORES)

    f32 = mybir.dt.float32
    bf16 = mybir.dt.bfloat16
    f8 = mybir.dt.float8e4
    DR = mybir.MatmulPerfMode.DoubleRow

    # x/c DRAM tensors are stored kt-major [kt, 128, node, batch] so that
    # kt-major SBUF tiles DMA with <=3 free dims.
    xt8 = nc.dram_tensor("xt8", [4, 128, N, BC], f8, kind="ExternalInput")
    xtb = nc.dram_tensor("xtb", [4, 128, N_TAIL, BC], bf16, kind="ExternalInput")
    w8 = nc.dram_tensor("w8", [NW, 128, 128], f8, kind="ExternalInput")
    wb = nc.dram_tensor("wb", [NW, 128, 128], bf16, kind="ExternalInput")
    b2_d = nc.dram_tensor("b2", [128, NJ], f32, kind="ExternalInput")
    bleaf_d = nc.dram_tensor("bleaf", [128, NJ], f32, kind="ExternalInput")
    b1_d = nc.dram_tensor("b1", [128, NJ], f32, kind="ExternalInput")

    # children c keyed by parent index t: CL[t] = c(2t+1), CR[t] = c(2t+2)
    CL = nc.dram_tensor("CLbuf", [4, 128, 128, BC], bf16)
    CR = nc.dram_tensor("CRbuf", [4, 128, 128, BC], bf16)

    c0t = nc.dram_tensor("c0t", [D, BC], f32, kind="ExternalOutput")
    h0t = nc.dram_tensor("h0t", [D, BC], f32, kind="ExternalOutput")

    # kt-major views [partition, ktile, node, batch]
    xt8_r = xt8.ap().rearrange("kt p n b -> p kt n b")
    xtb_r = xtb.ap().rearrange("kt p n b -> p kt n b")
    CL_r = CL.ap().rearrange("kt p t b -> p kt t b")
    CR_r = CR.ap().rearrange("kt p t b -> p kt t b")
    c0t_r = c0t.ap().rearrange("(kt p) b -> p kt b", p=128)
    h0t_r = h0t.ap().rearrange("(kt p) b -> p kt b", p=128)

    with tile.TileContext(nc) as tc:
        import contextlib

        ctx = contextlib.ExitStack()
        with ctx:
            wpool = ctx.enter_context(tc.tile_pool(name="wpool", bufs=1))
            hpool = ctx.enter_context(tc.tile_pool(name="hpool", bufs=1))
            inpool = ctx.enter_context(tc.tile_pool(name="inpool", bufs=2))
            gpool = ctx.enter_context(tc.tile_pool(name="gpool", bufs=2))
            epool = ctx.enter_context(tc.tile_pool(name="epool", bufs=2))
            pspool = ctx.enter_context(tc.tile_pool(name="ps", bufs=8, space="PSUM"))

            w8_sb = wpool.tile([128, NW, 128], f8, name="w8sb")
            nc.sync.dma_start(out=w8_sb[:], in_=w8.ap().rearrange("blk p c -> p blk c"))
            b2_sb = wpool.tile([128, NJ], f32, name="b2sb")
            bleaf_sb = wpool.tile([128, NJ], f32, name="bleafsb")
            b1_sb = wpool.tile([128, NJ], f32, name="b1sb")
            nc.sync.dma_start(out=b2_sb[:], in_=b2_d.ap()[:])
            nc.sync.dma_start(out=bleaf_sb[:], in_=bleaf_d.ap()[:])
            nc.sync.dma_start(out=b1_sb[:], in_=b1_d.ap()[:])
            # bf16 weights + tail x: only needed from L3 on; issued later so
            # they queue behind the leaf-phase input DMAs.
            wb_sb = wpool.tile([128, NW, 128], bf16, name="wbsb")
            xtb_t = wpool.tile([128, 4, N_TAIL, BC], bf16, name="xtb_t")

            def process(
                nodes,
                has_l,
                has_r,
                bias_sb,
                fp8_mode,
                child_h,  # (hL_tile, hR_tile, parent_base) or None
                out_h,  # (hL_cons, hR_cons, special) or None (root);
                # special: (tile, node) for a child outside the parent range
                child_c=None,  # (tile, base_node) -> children c from SBUF
                out_c=None,  # (tile, base_node) -> write c to SBUF, skip CL/CR
            ):
                """Compute (c, h) for `nodes` (a range), all at the same depth."""
                to_out = out_h is None
                for a in range(nodes.start, nodes.stop, 16):
                    b_ = min(a + 16, nodes.stop)
                    k = b_ - a  # nodes in this chunk
                    dt_g = f32 if to_out else bf16

                    if fp8_mode:
                        xt_t = inpool.tile([128, 4, k, BC], f8, name="xt_t")
                        nc.sync.dma_start(out=xt_t[:], in_=xt8_r[:, :, a:b_, :])
                    else:
                        xt_t = xtb_t[:, :, a : b_, :]
                    if child_c is None and child_h is not None:
                        if has_l:
                            cl_t = inpool.tile([128, 4, k, BC], bf16, name="cl_t")
                            nc.sync.dma_start(out=cl_t[:], in_=CL_r[:, :, a:b_, :])
                        if has_r:
                            cr_t = inpool.tile([128, 4, k, BC], bf16, name="cr_t")
                            nc.sync.dma_start(out=cr_t[:], in_=CR_r[:, :, a:b_, :])
                    if child_c is not None:
                        cc_t, cc_base = child_c
                        cs0 = 2 * a + 1 - cc_base
                        if k == 1:
                            cl_t = cc_t[:, :, cs0 : cs0 + 1, :]
                            cr_t = cc_t[:, :, cs0 + 1 : cs0 + 2, :]
                        else:
                            cl_t = cc_t[:, :, cs0 : cs0 + 2 * k - 1 : 2, :]
                            cr_t = cc_t[:, :, cs0 + 1 : cs0 + 2 * k : 2, :]
                    if child_h is not None:
                        chL, chR, ch_base = child_h
                        s = a - ch_base

                        def hl_slice(q0, q1):
                            return chL[:, q0:q1, s : s + k, :]

                        def hr_slice(q0, q1):
                            return chR[:, q0:q1, s : s + k, :]

                    g_i = gpool.tile([128, 4, k, BC], dt_g, name="g_i", bufs=1)
                    g_o = gpool.tile([128, 4, k, BC], dt_g, name="g_o")
                    g_u = gpool.tile([128, 4, k, BC], dt_g, name="g_u", bufs=1)
                    if has_l:
                        g_fl = gpool.tile([128, 4, k, BC], dt_g, name="g_fl", bufs=1)
                    if has_r:
                        g_fr = gpool.tile([128, 4, k, BC], dt_g, name="g_fr", bufs=1)

                    # f-gates first so the fl*cl / fr*cr muls overlap the
                    # i/u/o matmuls; o last, right before h = o * tanh(c).
                    js = []
                    if has_r:
                        js += list(range(16, 20))
                    if has_l:
                        js += list(range(12, 16))
                    js += list(range(0, 4)) + list(range(8, 12)) + list(range(4, 8))

                    for j in js:
                        kts = _used_kts(j, has_l, has_r)
                        ps = pspool.tile([128, k, BC], f32, name="ps")
                        if fp8_mode:
                            pairs = [(kts[q], kts[q + 1]) for q in range(0, len(kts), 2)]
                            for i, (kt0, kt1) in enumerate(pairs):
                                wi = widx(kt0, j)
                                assert widx(kt1, j) == wi + 1
                                if kt0 < 4:
                                    rhs = xt_t[:, kt0 : kt0 + 2, :, :]
                                elif kt0 < 8:
                                    rhs = hl_slice(kt0 - 4, kt0 - 2)
                                else:
                                    rhs = hr_slice(kt0 - 8, kt0 - 6)
                                nc.tensor.matmul(
                                    ps[:],
                                    w8_sb[:, wi : wi + 2, :],
                                    rhs,
                                    start=(i == 0),
                                    stop=(i == len(pairs) - 1),
                                    perf_mode=DR,
                                )
                        else:
                            for i, kt in enumerate(kts):
                                if kt < 4:
                                    rhs = xt_t[:, kt, :, :]
                                elif kt < 8:
                                    rhs = chL[:, kt - 4, s : s + k, :]
                                else:
                                    rhs = chR[:, kt - 8, s : s + k, :]
                                nc.tensor.matmul(
                                    ps[:],
                                    wb_sb[:, widx(kt, j), :],
                                    rhs,
                                    start=(i == 0),
                                    stop=(i == len(kts) - 1),
                                )
                        func = (
                            mybir.ActivationFunctionType.Tanh
                            if 8 <= j < 12
                            else mybir.ActivationFunctionType.Sigmoid
                        )
                        if j < 4:
                            dst = g_i[:, j, :, :]
                        elif j < 8:
                            dst = g_o[:, j - 4, :, :]
                        elif j < 12:
                            dst = g_u[:, j - 8, :, :]
                        elif j < 16:
                            dst = g_fl[:, j - 12, :, :]
                        else:
                            dst = g_fr[:, j - 16, :, :]
                        nc.scalar.activation(
                            out=dst,
                            in_=ps[:],
                            func=func,
                            bias=bias_sb[:, j : j + 1],
                            scale=(1.0 / WSCALE) if fp8_mode else 1.0,
                        )

                    # c = i*u (+ fl*cl) (+ fr*cr);  h = o * tanh(c)
                    if out_c is not None:
                        oc_t, oc_base = out_c
                        c_t = oc_t[:, :, a - oc_base : b_ - oc_base, :]
                    else:
                        c_t = epool.tile([128, 4, k, BC], dt_g, name="c_t")
                    if has_r:
                        m3 = epool.tile([128, 4, k, BC], dt_g, name="m3", bufs=1)
                        eng = nc.gpsimd if k == 16 else nc.vector
                        eng.tensor_mul(m3[:], g_fr[:], cr_t[:])
                    if has_l:
                        m2 = epool.tile([128, 4, k, BC], dt_g, name="m2", bufs=1)
                        nc.vector.tensor_mul(m2[:], g_fl[:], cl_t[:])
                    nc.vector.tensor_mul(c_t[:], g_i[:], g_u[:])
                    if has_l:
                        nc.vector.tensor_add(c_t[:], c_t[:], m2[:])
                    if has_r:
                        nc.vector.tensor_add(c_t[:], c_t[:], m3[:])
                    tc_t = epool.tile([128, 4, k, BC], dt_g, name="tc_t")
                    nc.scalar.activation(
                        out=tc_t[:], in_=c_t[:], func=mybir.ActivationFunctionType.Tanh
                    )

                    if to_out:
                        h_t = epool.tile([128, 4, k, BC], dt_g, name="h_t")
                        nc.vector.tensor_mul(h_t[:], g_o[:], tc_t[:])
                        nc.sync.dma_start(out=c0t_r[:], in_=c_t[:, :, 0, :])
                        nc.sync.dma_start(out=h0t_r[:], in_=h_t[:, :, 0, :])
                        continue

                    # h: odd nodes -> left-child slot of parent, evens -> right
                    hL_cons, hR_cons, special = out_h
                    odd0 = 0 if a % 2 == 1 else 1
                    even0 = 1 - odd0
                    odds = list(range(a + odd0, b_, 2))
                    evens = list(range(a + even0, b_, 2))
                    if special is not None:
                        sp_tile, sp_node = special
                        if sp_node in odds:
                            odds.remove(sp_node)
                            io = sp_node - a
                            nc.vector.tensor_mul(
                                sp_tile[:, :, 0:1, :],
                                g_o[:, :, io : io + 1, :],
                                tc_t[:, :, io : io + 1, :],
                            )
                    if odds:
                        tile_, base = hL_cons
                        lo = (odds[0] - 1) // 2 - base
                        nc.vector.tensor_mul(
                            tile_[:, :, lo : lo + len(odds), :],
                            g_o[:, :, odd0 : odd0 + 2 * len(odds) - 1 : 2, :],
                            tc_t[:, :, odd0 : odd0 + 2 * len(odds) - 1 : 2, :],
                        )
                    if evens:
                        tile_, base = hR_cons
                        ro = evens[0] // 2 - 1 - base
                        nc.vector.tensor_mul(
                            tile_[:, :, ro : ro + len(evens), :],
                            g_o[:, :, even0 : even0 + 2 * len(evens) - 1 : 2, :],
                            tc_t[:, :, even0 : even0 + 2 * len(evens) - 1 : 2, :],
                        )

                    if out_c is not None:
                        continue  # c already written to its SBUF level tile
                    # c of node t -> CL[(t-1)//2] if t odd else CR[t//2 - 1]
                    # (per-kt DMAs: the stride-2 node slice + kt dim exceeds
                    # the 3-dim DMA AP limit otherwise)
                    all_odds = list(range(a + odd0, b_, 2))
                    for kt in range(4):
                        if all_odds:
                            lo = (all_odds[0] - 1) // 2
                            nc.sync.dma_start(
                                out=CL_r[:, kt, lo : lo + len(all_odds), :],
                                in_=c_t[:, kt, odd0::2, :],
                            )
                        if evens:
                            ro = evens[0] // 2 - 1
                            nc.sync.dma_start(
                                out=CR_r[:, kt, ro : ro + len(evens), :],
                                in_=c_t[:, kt, even0::2, :],
                            )

            # c stays in SBUF for the small tail levels (outputs of L4..L1);
            # their parent phases then skip the CL/CR DRAM roundtrip entirely.
            C_SBUF_LVLS = (4, 3, 2, 1)
            FP8_PHASES = {7, 127, 6, 5, 4}  # leaves, node127, L6, L5, L4

            for _rep in range(reps):
                # per-consumer-level h tiles (SBUF-resident), parity-split.
                # hL[lvl]/hR[lvl] feed the phase processing level `lvl`:
                # slot (t - base(lvl)) holds h of left/right child of node t.
                hL, hR = {}, {}
                for lvl in range(7):
                    n_lvl = 2**lvl
                    dt_h = f8 if lvl in FP8_PHASES else bf16
                    hL[lvl] = (
                        hpool.tile([128, 4, n_lvl, BC], dt_h, name=f"hL_{lvl}"),
                        n_lvl - 1,
                    )
                    hR[lvl] = (
                        hpool.tile([128, 4, n_lvl, BC], dt_h, name=f"hR_{lvl}"),
                        n_lvl - 1,
                    )
                hX127 = hpool.tile([128, 4, 1, BC], f8, name="hX127")  # h of node 255
                lvl_c = {}
                for lvl in C_SBUF_LVLS:
                    tl = hpool.tile([128, 4, 2**lvl, BC], bf16, name=f"c_{lvl}")
                    lvl_c[lvl] = (tl, 2**lvl - 1)

                # leaves: nodes 128..255 (no children). Node 255 -> hX127.
                process(
                    range(128, 256),
                    False,
                    False,
                    bleaf_sb,
                    True,
                    None,
                    (hL[6], hR[6], (hX127, 255)),
                )
                # node 127: left child only (node 255 in hX127); h -> hL6 slot 0
                process(
                    range(127, 128),
                    True,
                    False,
                    b1_sb,
                    True,
                    (hX127, None, 127),
                    (hL[6], hR[6], None),
                )
                if reps == 1 or _rep == 0:
                    # bf16 tail weights / x: queue after leaf-phase DMAs
                    nc.sync.dma_start(
                        out=wb_sb[:], in_=wb.ap().rearrange("blk p c -> p blk c")
                    )
                    nc.sync.dma_start(out=xtb_t[:], in_=xtb_r[:, :, :, :])
                # levels 6..1: two children each
                for lvl in range(6, 0, -1):
                    process(
                        range(2**lvl - 1, 2 ** (lvl + 1) - 1),
                        True,
                        True,
                        b2_sb,
                        lvl in FP8_PHASES,
                        (hL[lvl][0], hR[lvl][0], hL[lvl][1]),
                        (hL[lvl - 1], hR[lvl - 1], None),
                        child_c=lvl_c.get(lvl + 1),
                        out_c=lvl_c.get(lvl),
                    )
                # root
                process(
                    range(0, 1),
                    True,
                    True,
                    b2_sb,
                    False,
                    (hL[0][0], hR[0][0], 0),
                    None,
                    child_c=lvl_c.get(1),
                )

    nc.compile()
    return nc


def _expected_tree():
    left = np.array([2 * i + 1 if 2 * i + 1 < N else 0 for i in range(N)], np.int32)
    right = np.array([2 * i + 2 if 2 * i + 2 < N else 0 for i in range(N)], np.int32)
    nch = np.array(
        [int(2 * i + 1 < N) + int(2 * i + 2 < N) for i in range(N)], np.int32
    )
    return left, right, nch


def _pack_w_big(W_ioux, W_fx, W_iouhL, W_fhL, W_iouhR, W_fhR):
    w_big = np.zeros((1536, 2560), np.float32)
    w_big[0:512, 0:1536] = np.asarray(W_ioux, np.float32).T
    w_big[0:512, 1536:2048] = np.asarray(W_fx, np.float32).T
    w_big[0:512, 2048:2560] = np.asarray(W_fx, np.float32).T
    w_big[512:1024, 0:1536] = np.asarray(W_iouhL, np.float32).T
    w_big[512:1024, 1536:2048] = np.asarray(W_fhL, np.float32).T
    w_big[1024:1536, 0:1536] = np.asarray(W_iouhR, np.float32).T
    w_big[1024:1536, 2048:2560] = np.asarray(W_fhR, np.float32).T
    w_np = np.empty((NW, 128, 128), np.float32)
    for i, (kt, j) in enumerate(W_BLOCKS):
        w_np[i] = w_big[kt * 128 : (kt + 1) * 128, j * 128 : (j + 1) * 128]
    return np.ascontiguousarray(w_np)


def pack_biases(b_ioux, b_iouh, b_iouhL, b_iouhR, b_fx, b_fhL, b_fhR):
    def pack(vec):
        return np.ascontiguousarray(np.asarray(vec, np.float32).reshape(NJ, 128).T)

    z = np.zeros(512, np.float32)
    b2 = pack(np.concatenate([b_ioux + b_iouhL + b_iouhR, b_fx + b_fhL, b_fx + b_fhR]))
    bleaf = pack(np.concatenate([b_ioux + b_iouh, z, z]))
    b1 = pack(np.concatenate([b_ioux + b_iouhL, b_fx + b_fhL, z]))
    return b2, bleaf, b1


def prepare_in_maps(np_inputs):
    i = np_inputs
    inputs = np.asarray(i["inputs"], np.float32)
    w_f32 = _pack_w_big(
        i["W_ioux"], i["W_fx"], i["W_iouhL"], i["W_fhL"], i["W_iouhR"], i["W_fhR"]
    )
    w8_np = (w_f32 * WSCALE).astype(FP8)
    wb_np = w_f32.astype(BF16)
    b_args = [
        np.asarray(i[k], np.float32)
        for k in ("b_ioux", "b_iouh", "b_iouhL", "b_iouhR", "b_fx", "b_fhL", "b_fhR")
    ]
    b2, bleaf, b1 = pack_biases(*b_args)

    in_maps = []
    for c in range(NCORES):
        xc = inputs[c * BC : (c + 1) * BC]  # [BC, N, D]
        # [N, D, BC] -> kt-major [4, 128, N, BC]
        xt_c = xc.transpose(1, 2, 0).reshape(N, 4, 128, BC).transpose(1, 2, 0, 3)
        xt_c = np.ascontiguousarray(xt_c)
        in_maps.append(
            {
                "xt8": xt_c.astype(FP8),
                "xtb": np.ascontiguousarray(xt_c[:, :, :N_TAIL]).astype(BF16),
                "w8": w8_np,
                "wb": wb_np,
                "b2": b2,
                "bleaf": bleaf,
                "b1": b1,
            }
        )
    return in_maps


def kernel(
    inputs,
    W_ioux, b_ioux, W_iouh, b_iouh, W_iouhL, b_iouhL, W_iouhR, b_iouhR,
    W_fx, b_fx, W_fh, b_fh, W_fhL, b_fhL, W_fhR, b_fhR,
    left_idx, right_idx, num_children,
):
    el, er, en = _expected_tree()
    assert np.array_equal(np.asarray(left_idx), el), "unexpected tree structure"
    assert np.array_equal(np.asarray(right_idx), er), "unexpected tree structure"
    assert np.array_equal(np.asarray(num_children), en), "unexpected tree structure"

    in_maps = prepare_in_maps(
        dict(
            inputs=inputs,
            W_ioux=W_ioux, W_fx=W_fx, W_iouhL=W_iouhL, W_fhL=W_fhL,
            W_iouhR=W_iouhR, W_fhR=W_fhR,
            b_ioux=b_ioux, b_iouh=b_iouh, b_iouhL=b_iouhL, b_iouhR=b_iouhR,
            b_fx=b_fx, b_fhL=b_fhL, b_fhR=b_fhR,
        )
    )

    if "nc" not in _compiled:
        _compiled["nc"] = _build_bass()
    nc = _compiled["nc"]

    res = run_bass_kernel_spmd(
        nc, in_maps, core_ids=list(range(NCORES)), trace=bool(_compiled.get("trace"))
    )
    _compiled["last_res"] = res

    c_full = np.empty((B, D), np.float32)
    h_full = np.empty((B, D), np.float32)
    for c in range(NCORES):
        c_full[c * BC : (c + 1) * BC] = res.results[c]["c0t"].T
        h_full[c * BC : (c + 1) * BC] = res.results[c]["h0t"].T
    return c_full, h_full


# revision 39
# speedup vs baseline: 1.9248x; 1.2509x over previous
"""ConstituencyTreeLSTM Trainium2 kernel (fp8 DoubleRow edition).

Strategy:
  - Data-parallel over the B=256 batch across 8 NeuronCores (32 rows/core).
  - Complete-heap tree -> level-parallel phases:
      leaves (128..255) -> node 127 -> L6 (63..126) -> ... -> L1 -> root.
  - Feature-on-partitions layout; matmul PSUM outputs feed the next level
    without transposes. All SBUF tiles are kt-major: [128, kt, node, batch].
  - Big levels (leaves, 127, L6, L5, L4) run matmuls in fp8e4 with
    MatmulPerfMode.DoubleRow: 2 k-tiles contracted per instruction at
    0.5 cycles/row -> 4x bf16 PE throughput, half the instructions.
    Weights are scaled x64 host-side (keeps uniform(+-1/sqrt(512)) weights
    in the fp8 normal range); the 1/64 is folded into the PSUM-evacuating
    activation's scale. x and h are quantized to fp8 unscaled.
  - Small levels (L3..root, 15 nodes) run in bf16: their PE time is
    negligible and this restores most of the accuracy (sim: 1.3e-2 vs
    3.7e-2 all-fp8, threshold 2e-2).
  - h lives in SBUF per level, parity-split by parent (left-children tile /
    right-children tile) so DoubleRow rhs slices stay dense: the stride-2
    child gather becomes a contiguous slice indexed by parent.
  - c of big levels goes through DRAM (CL/CR, parity-split by parent);
    c of L4..L1 stays in SBUF.
  - Per-node-type biases (2-child / leaf / 1-child) folded host-side,
    applied inside the PSUM-evacuating activation (sigmoid/tanh).
  - Elementwise c/h stage on DVE (2x bf16) with one mul offloaded to Pool.
"""

import sys

sys.path.insert(0, "/opt/trn_rl_repo")

import numpy as np
import ml_dtypes

import concourse.bass as bass  # noqa: F401
import concourse.mybir as mybir
import concourse.tile as tile
from concourse import bacc
from concourse.bass_utils import run_bass_kernel_spmd

BF16 = ml_dtypes.bfloat16
FP8 = ml_dtypes.float8_e4m3
NCORES = 8
B, N, D = 256, 256, 512
BC = B // NCORES  # batch rows per core
NJ = 20  # output j-tiles: 12 iou + 4 fL + 4 fR
WSCALE = 64.0  # fp8 weight scale; inverse folded into activation scale

N_TAIL = 15  # nodes 0..14 (L3..root) run in bf16

_compiled = {}


def _used_kts(j, has_l=True, has_r=True):
    if j < 12:
        kts = list(range(0, 4)) + (list(range(4, 8)) if has_l else []) + (
            list(range(8, 12)) if has_r else []
        )
    elif j < 16:
        kts = list(range(0, 4)) + list(range(4, 8))
    else:
        kts = list(range(0, 4)) + list(range(8, 12))
    return kts


# packed weight-block index: only (kt, j) pairs with nonzero weight blocks.
# The x-blocks of j16..19 (fR's W_fx) duplicate j12..15's (fL's W_fx) and are
# not stored; widx() remaps them. Leaf-phase blocks (x-kts of the iou gates)
# come first so the leaf matmuls only wait on a small initial weight DMA.
W_LEAF = [(kt, j) for j in range(12) for kt in range(4)]
# rest ordered so node-127's blocks (hL of iou + all of fL) come first
W_REST = (
    [(kt, j) for j in range(12) for kt in range(4, 8)]
    + [(kt, j) for j in range(12, 16) for kt in range(8)]
    + [(kt, j) for j in range(12) for kt in range(8, 12)]
    + [(kt, j) for j in range(16, 20) for kt in range(8, 12)]
)
W_BLOCKS = W_LEAF + W_REST
NW_LEAF = len(W_LEAF)  # 48
W_IDX = {p: i for i, p in enumerate(W_BLOCKS)}
NW = len(W_BLOCKS)  # 192


def widx(kt, j):
    if j >= 16 and kt < 4:
        j = j - 4
    return W_IDX[(kt, j)]


def _build_bass(reps=1, skip_ew=False, skip_act=False):
    nc = bacc.Bacc("TRN2", target_bir_lowering=False, debug=False, num_devices=NCORES)

    f32 = mybir.dt.float32
    bf16 = mybir.dt.bfloat16
    f8 = mybir.dt.float8e4
    DR = mybir.MatmulPerfMode.DoubleRow

    # x/c DRAM tensors are stored kt-major [kt, 128, node, batch] so that
    # kt-major SBUF tiles DMA with <=3 free dims.
    xt8 = nc.dram_tensor("xt8", [4, 128, N, BC], f8, kind="ExternalInput")
    xtb = nc.dram_tensor("xtb", [4, 128, N_TAIL, BC], bf16, kind="ExternalInput")
    w8 = nc.dram_tensor("w8", [NW, 128, 128], f8, kind="ExternalInput")
    wb = nc.dram_tensor("wb", [NW, 128, 128], bf16, kind="ExternalInput")
    b2_d = nc.dram_tensor("b2", [128, NJ], f32, kind="ExternalInput")
    bleaf_d = nc.dram_tensor("bleaf", [128, NJ], f32, kind="ExternalInput")
    b1_d = nc.dram_tensor("b1", [128, NJ], f32, kind="ExternalInput")

    # children c keyed by parent index t: CL[t] = c(2t+1), CR[t] = c(2t+2)
    CL = nc.dram_tensor("CLbuf", [4, 128, 128, BC], bf16)
    CR = nc.dram_tensor("CRbuf", [4, 128, 128, BC], bf16)

    c0t = nc.dram_tensor("c0t", [D, BC], f32, kind="ExternalOutput")
    h0t = nc.dram_tensor("h0t", [D, BC], f32, kind="ExternalOutput")

    # kt-major views [partition, ktile, node, batch]
    xt8_r = xt8.ap().rearrange("kt p n b -> p kt n b")
    xtb_r = xtb.ap().rearrange("kt p n b -> p kt n b")
    CL_r = CL.ap().rearrange("kt p t b -> p kt t b")
    CR_r = CR.ap().rearrange("kt p t b -> p kt t b")
    c0t_r = c0t.ap().rearrange("(kt p) b -> p kt b", p=128)
    h0t_r = h0t.ap().rearrange("(kt p) b -> p kt b", p=128)

    with tile.TileContext(nc) as tc:
        import contextlib

        ctx = contextlib.ExitStack()
        with ctx:
            wpool = ctx.enter_context(tc.tile_pool(name="wpool", bufs=1))
            hpool = ctx.enter_context(tc.tile_pool(name="hpool", bufs=1))
            inpool = ctx.enter_context(tc.tile_pool(name="inpool", bufs=2))
            gpool = ctx.enter_context(tc.tile_pool(name="gpool", bufs=2))
            epool = ctx.enter_context(tc.tile_pool(name="epool", bufs=2))
            pspool = ctx.enter_context(tc.tile_pool(name="ps", bufs=8, space="PSUM"))

            w8_sb = wpool.tile([128, NW, 128], f8, name="w8sb")
            w8_r = w8.ap().rearrange("blk p c -> p blk c")
            # leaf-phase blocks first; everything else is queued as small
            # pieces interleaved between per-chunk loads (drained inside
            # process()) so no big transfer ever blocks a chunk load.
            nc.sync.dma_start(out=w8_sb[:, :NW_LEAF, :], in_=w8_r[:, :NW_LEAF, :])
            b2_sb = wpool.tile([128, NJ], f32, name="b2sb")
            bleaf_sb = wpool.tile([128, NJ], f32, name="bleafsb")
            b1_sb = wpool.tile([128, NJ], f32, name="b1sb")
            nc.sync.dma_start(out=b2_sb[:], in_=b2_d.ap()[:])
            nc.sync.dma_start(out=bleaf_sb[:], in_=bleaf_d.ap()[:])
            nc.sync.dma_start(out=b1_sb[:], in_=b1_d.ap()[:])
            wb_sb = wpool.tile([128, NW, 128], bf16, name="wbsb")
            xtb_t = wpool.tile([128, 4, N_TAIL, BC], bf16, name="xtb_t")
            wb_r = wb.ap().rearrange("blk p c -> p blk c")

            pending = []
            for lo_, hi_ in [(48, 96), (96, 128), (128, 176), (176, 192)]:
                pending.append(
                    lambda lo=lo_, hi=hi_: nc.sync.dma_start(
                        out=w8_sb[:, lo:hi, :], in_=w8_r[:, lo:hi, :]
                    )
                )
            for lo_ in range(0, NW, 32):
                hi_ = min(lo_ + 32, NW)
                pending.append(
                    lambda lo=lo_, hi=hi_: nc.sync.dma_start(
                        out=wb_sb[:, lo:hi, :], in_=wb_r[:, lo:hi, :]
                    )
                )
            pending.append(lambda: nc.sync.dma_start(out=xtb_t[:], in_=xtb_r[:]))

            def drain_pending(n=1):
                for _ in range(n):
                    if pending:
                        pending.pop(0)()

            def process(
                nodes,
                has_l,
                has_r,
                bias_sb,
                mode,  # 'fp8' | 'mix' (bf16 x, fp8 h) | 'bf16'
                child_h,  # (hL_tile, hR_tile, parent_base) or None
                out_h,  # (hL_cons, hR_cons, special) or None (root);
                # special: (tile, node) for a child outside the parent range
                child_c=None,  # (tile, base_node) -> children c from SBUF
                out_c=None,  # (tile, base_node) -> write c to SBUF, skip CL/CR
                chunk_first=None,  # chunk start to hoist to the front
            ):
                """Compute (c, h) for `nodes` (a range), all at the same depth."""
                to_out = out_h is None
                starts = list(range(nodes.start, nodes.stop, 16))
                if chunk_first is not None and chunk_first in starts:
                    starts.remove(chunk_first)
                    starts.insert(0, chunk_first)
                for a in starts:
                    b_ = min(a + 16, nodes.stop)
                    k = b_ - a  # nodes in this chunk
                    dt_g = f32 if to_out else bf16

                    if mode == "fp8":
                        xt_t = inpool.tile([128, 4, k, BC], f8, name="xt_t")
                        nc.sync.dma_start(out=xt_t[:], in_=xt8_r[:, :, a:b_, :])
                    else:
                        xt_t = xtb_t[:, :, a : b_, :]
                    drain_pending()
                    if child_c is None and child_h is not None:
                        if has_l:
                            cl_t = inpool.tile([128, 4, k, BC], bf16, name="cl_t")
                            nc.sync.dma_start(out=cl_t[:], in_=CL_r[:, :, a:b_, :])
                        if has_r:
                            cr_t = inpool.tile([128, 4, k, BC], bf16, name="cr_t")
                            nc.sync.dma_start(out=cr_t[:], in_=CR_r[:, :, a:b_, :])
                    if child_c is not None:
                        cc_t, cc_base = child_c
                        cs0 = 2 * a + 1 - cc_base
                        if k == 1:
                            cl_t = cc_t[:, :, cs0 : cs0 + 1, :]
                            cr_t = cc_t[:, :, cs0 + 1 : cs0 + 2, :]
                        else:
                            cl_t = cc_t[:, :, cs0 : cs0 + 2 * k - 1 : 2, :]
                            cr_t = cc_t[:, :, cs0 + 1 : cs0 + 2 * k : 2, :]
                    if child_h is not None:
                        chL, chR, ch_base = child_h
                        s = a - ch_base

                        def hl_slice(q0, q1):
                            return chL[:, q0:q1, s : s + k, :]

                        def hr_slice(q0, q1):
                            return chR[:, q0:q1, s : s + k, :]

                    g_i = gpool.tile([128, 4, k, BC], dt_g, name="g_i", bufs=1)
                    g_o = gpool.tile([128, 4, k, BC], dt_g, name="g_o")
                    g_u = gpool.tile([128, 4, k, BC], dt_g, name="g_u", bufs=1)
                    if has_l:
                        g_fl = gpool.tile([128, 4, k, BC], dt_g, name="g_fl", bufs=1)
                    if has_r:
                        g_fr = gpool.tile([128, 4, k, BC], dt_g, name="g_fr", bufs=1)

                    # f-gates first so the fl*cl / fr*cr muls overlap the
                    # i/u/o matmuls; o last, right before h = o * tanh(c).
                    js = []
                    if has_r:
                        js += list(range(16, 20))
                    if has_l:
                        js += list(range(12, 16))
                    js += list(range(0, 4)) + list(range(8, 12)) + list(range(4, 8))

                    for j in js:
                        kts = _used_kts(j, has_l, has_r)
                        ps = pspool.tile([128, k, BC], f32, name="ps")
                        if mode == "fp8":
                            pairs = [(kts[q], kts[q + 1]) for q in range(0, len(kts), 2)]
                            for i, (kt0, kt1) in enumerate(pairs):
                                wi = widx(kt0, j)
                                assert widx(kt1, j) == wi + 1
                                if kt0 < 4:
                                    rhs = xt_t[:, kt0 : kt0 + 2, :, :]
                                elif kt0 < 8:
                                    rhs = hl_slice(kt0 - 4, kt0 - 2)
                                else:
                                    rhs = hr_slice(kt0 - 8, kt0 - 6)
                                nc.tensor.matmul(
                                    ps[:],
                                    w8_sb[:, wi : wi + 2, :],
                                    rhs,
                                    start=(i == 0),
                                    stop=(i == len(pairs) - 1),
                                    perf_mode=DR,
                                )
                        elif mode == "mix":
                            # bf16 x singles + fp8 DoubleRow h pairs, one group
                            hkts = [kt for kt in kts if kt >= 4]
                            pairs = [
                                (hkts[q], hkts[q + 1]) for q in range(0, len(hkts), 2)
                            ]
                            for i, kt in enumerate(range(4)):
                                nc.tensor.matmul(
                                    ps[:],
                                    wb_sb[:, widx(kt, j), :],
                                    xt_t[:, kt, :, :],
                                    start=(i == 0),
                                    stop=False,
                                )
                            for i, (kt0, kt1) in enumerate(pairs):
                                wi = widx(kt0, j)
                                assert widx(kt1, j) == wi + 1
                                if kt0 < 8:
                                    rhs = hl_slice(kt0 - 4, kt0 - 2)
                                else:
                                    rhs = hr_slice(kt0 - 8, kt0 - 6)
                                nc.tensor.matmul(
                                    ps[:],
                                    w8_sb[:, wi : wi + 2, :],
                                    rhs,
                                    start=False,
                                    stop=(i == len(pairs) - 1),
                                    perf_mode=DR,
                                )
                        else:
                            for i, kt in enumerate(kts):
                                if kt < 4:
                                    rhs = xt_t[:, kt, :, :]
                                elif kt < 8:
                                    rhs = chL[:, kt - 4, s : s + k, :]
                                else:
                                    rhs = chR[:, kt - 8, s : s + k, :]
                                nc.tensor.matmul(
                                    ps[:],
                                    wb_sb[:, widx(kt, j), :],
                                    rhs,
                                    start=(i == 0),
                                    stop=(i == len(kts) - 1),
                                )
                        func = (
                            mybir.ActivationFunctionType.Tanh
                            if 8 <= j < 12
                            else mybir.ActivationFunctionType.Sigmoid
                        )
                        if j < 4:
                            dst = g_i[:, j, :, :]
                        elif j < 8:
                            dst = g_o[:, j - 4, :, :]
                        elif j < 12:
                            dst = g_u[:, j - 8, :, :]
                        elif j < 16:
                            dst = g_fl[:, j - 12, :, :]
                        else:
                            dst = g_fr[:, j - 16, :, :]
                        if not skip_act:
                            # both w8 and wb carry the x64 scale
                            nc.scalar.activation(
                                out=dst,
                                in_=ps[:],
                                func=func,
                                bias=bias_sb[:, j : j + 1],
                                scale=1.0 / WSCALE,
                            )

                    if skip_ew or skip_act:
                        continue
                    # c = i*u (+ fl*cl) (+ fr*cr);  h = o * tanh(c)
                    if out_c is not None:
                        oc_t, oc_base = out_c
                        c_t = oc_t[:, :, a - oc_base : b_ - oc_base, :]
                    else:
                        c_t = epool.tile([128, 4, k, BC], dt_g, name="c_t")
                    if has_r:
                        m3 = epool.tile([128, 4, k, BC], dt_g, name="m3", bufs=1)
                        eng = nc.gpsimd if k == 16 else nc.vector
                        eng.tensor_mul(m3[:], g_fr[:], cr_t[:])
                    if has_l:
                        m2 = epool.tile([128, 4, k, BC], dt_g, name="m2", bufs=1)
                        nc.vector.tensor_mul(m2[:], g_fl[:], cl_t[:])
                    nc.vector.tensor_mul(c_t[:], g_i[:], g_u[:])
                    if has_l:
                        nc.vector.tensor_add(c_t[:], c_t[:], m2[:])
                    if has_r:
                        nc.vector.tensor_add(c_t[:], c_t[:], m3[:])
                    tc_t = epool.tile([128, 4, k, BC], dt_g, name="tc_t")
                    nc.scalar.activation(
                        out=tc_t[:], in_=c_t[:], func=mybir.ActivationFunctionType.Tanh
                    )

                    if to_out:
                        h_t = epool.tile([128, 4, k, BC], dt_g, name="h_t")
                        nc.vector.tensor_mul(h_t[:], g_o[:], tc_t[:])
                        nc.sync.dma_start(out=c0t_r[:], in_=c_t[:, :, 0, :])
                        nc.sync.dma_start(out=h0t_r[:], in_=h_t[:, :, 0, :])
                        continue

                    # h: odd nodes -> left-child slot of parent, evens -> right
                    hL_cons, hR_cons, special = out_h
                    odd0 = 0 if a % 2 == 1 else 1
                    even0 = 1 - odd0
                    odds = list(range(a + odd0, b_, 2))
                    evens = list(range(a + even0, b_, 2))
                    if special is not None:
                        sp_tile, sp_node = special
                        if sp_node in odds:
                            odds.remove(sp_node)
                            io = sp_node - a
                            nc.vector.tensor_mul(
                                sp_tile[:, :, 0:1, :],
                                g_o[:, :, io : io + 1, :],
                                tc_t[:, :, io : io + 1, :],
                            )
                    if odds:
                        tile_, base = hL_cons
                        lo = (odds[0] - 1) // 2 - base
                        nc.vector.tensor_mul(
                            tile_[:, :, lo : lo + len(odds), :],
                            g_o[:, :, odd0 : odd0 + 2 * len(odds) - 1 : 2, :],
                            tc_t[:, :, odd0 : odd0 + 2 * len(odds) - 1 : 2, :],
                        )
                    if evens:
                        tile_, base = hR_cons
                        ro = evens[0] // 2 - 1 - base
                        nc.vector.tensor_mul(
                            tile_[:, :, ro : ro + len(evens), :],
                            g_o[:, :, even0 : even0 + 2 * len(evens) - 1 : 2, :],
                            tc_t[:, :, even0 : even0 + 2 * len(evens) - 1 : 2, :],
                        )

                    if out_c is not None:
                        continue  # c already written to its SBUF level tile
                    # c of node t -> CL[(t-1)//2] if t odd else CR[t//2 - 1]
                    # (per-kt DMAs: the stride-2 node slice + kt dim exceeds
                    # the 3-dim DMA AP limit otherwise)
                    all_odds = list(range(a + odd0, b_, 2))
                    for kt in range(4):
                        if all_odds:
                            lo = (all_odds[0] - 1) // 2
                            nc.sync.dma_start(
                                out=CL_r[:, kt, lo : lo + len(all_odds), :],
                                in_=c_t[:, kt, odd0::2, :],
                            )
                        if evens:
                            ro = evens[0] // 2 - 1
                            nc.sync.dma_start(
                                out=CR_r[:, kt, ro : ro + len(evens), :],
                                in_=c_t[:, kt, even0::2, :],
                            )

            # c stays in SBUF for the small tail levels (outputs of L4..L1);
            # their parent phases then skip the CL/CR DRAM roundtrip entirely.
            C_SBUF_LVLS = (4, 3, 2, 1)
            # per-phase precision: big levels full fp8; L3/L2 keep x in bf16
            # (the accuracy-sensitive path) with fp8 DoubleRow h; L1+root bf16
            PH_MODE = {
                7: "fp8", 6: "fp8", 5: "fp8", 4: "fp8",
                3: "mix", 2: "mix", 1: "bf16", 0: "bf16",
            }

            for _rep in range(reps):
                # per-consumer-level h tiles (SBUF-resident), parity-split.
                # hL[lvl]/hR[lvl] feed the phase processing level `lvl`:
                # slot (t - base(lvl)) holds h of left/right child of node t.
                hL, hR = {}, {}
                for lvl in range(7):
                    n_lvl = 2**lvl
                    dt_h = f8 if PH_MODE[lvl] in ("fp8", "mix") else bf16
                    hL[lvl] = (
                        hpool.tile([128, 4, n_lvl, BC], dt_h, name=f"hL_{lvl}"),
                        n_lvl - 1,
                    )
                    hR[lvl] = (
                        hpool.tile([128, 4, n_lvl, BC], dt_h, name=f"hR_{lvl}"),
                        n_lvl - 1,
                    )
                hX127 = hpool.tile([128, 4, 1, BC], f8, name="hX127")  # h of node 255
                lvl_c = {}
                for lvl in C_SBUF_LVLS:
                    tl = hpool.tile([128, 4, 2**lvl, BC], bf16, name=f"c_{lvl}")
                    lvl_c[lvl] = (tl, 2**lvl - 1)

                if skip_ew or skip_act:
                    # ablation modes never write h/c tiles; zero them so the
                    # tile framework sees a writer for every read
                    for t, _ in list(hL.values()) + list(hR.values()):
                        nc.vector.memset(t[:], 0.0)
                    nc.vector.memset(hX127[:], 0.0)
                    for t, _ in lvl_c.values():
                        nc.vector.memset(t[:], 0.0)

                # leaves: nodes 128..255 (no children). Node 255 -> hX127.
                # The chunk holding node 255 runs first: node 127 (and through
                # it all of L6) depends on it.
                process(
                    range(128, 256),
                    False,
                    False,
                    bleaf_sb,
                    "fp8",
                    None,
                    (hL[6], hR[6], (hX127, 255)),
                    chunk_first=240,
                )
                # node 127: left child only (node 255 in hX127); h -> hL6 slot 0
                process(
                    range(127, 128),
                    True,
                    False,
                    b1_sb,
                    "fp8",
                    (hX127, None, 127),
                    (hL[6], hR[6], None),
                )
                # levels 6..1: two children each
                for lvl in range(6, 0, -1):
                    process(
                        range(2**lvl - 1, 2 ** (lvl + 1) - 1),
                        True,
                        True,
                        b2_sb,
                        PH_MODE[lvl],
                        (hL[lvl][0], hR[lvl][0], hL[lvl][1]),
                        (hL[lvl - 1], hR[lvl - 1], None),
                        child_c=lvl_c.get(lvl + 1),
                        out_c=lvl_c.get(lvl),
                    )
                # root
                process(
                    range(0, 1),
                    True,
                    True,
                    b2_sb,
                    PH_MODE[0],
                    (hL[0][0], hR[0][0], 0),
                    None,
                    child_c=lvl_c.get(1),
                )

    nc.compile()
    return nc


def _expected_tree():
    left = np.array([2 * i + 1 if 2 * i + 1 < N else 0 for i in range(N)], np.int32)
    right = np.array([2 * i + 2 if 2 * i + 2 < N else 0 for i in range(N)], np.int32)
    nch = np.array(
        [int(2 * i + 1 < N) + int(2 * i + 2 < N) for i in range(N)], np.int32
    )
    return left, right, nch


def _pack_w_big(W_ioux, W_fx, W_iouhL, W_fhL, W_iouhR, W_fhR):
    w_big = np.zeros((1536, 2560), np.float32)
    w_big[0:512, 0:1536] = np.asarray(W_ioux, np.float32).T
    w_big[0:512, 1536:2048] = np.asarray(W_fx, np.float32).T
    w_big[0:512, 2048:2560] = np.asarray(W_fx, np.float32).T
    w_big[512:1024, 0:1536] = np.asarray(W_iouhL, np.float32).T
    w_big[512:1024, 1536:2048] = np.asarray(W_fhL, np.float32).T
    w_big[1024:1536, 0:1536] = np.asarray(W_iouhR, np.float32).T
    w_big[1024:1536, 2048:2560] = np.asarray(W_fhR, np.float32).T
    w_np = np.empty((NW, 128, 128), np.float32)
    for i, (kt, j) in enumerate(W_BLOCKS):
        w_np[i] = w_big[kt * 128 : (kt + 1) * 128, j * 128 : (j + 1) * 128]
    return np.ascontiguousarray(w_np)


def pack_biases(b_ioux, b_iouh, b_iouhL, b_iouhR, b_fx, b_fhL, b_fhR):
    def pack(vec):
        return np.ascontiguousarray(np.asarray(vec, np.float32).reshape(NJ, 128).T)

    z = np.zeros(512, np.float32)
    b2 = pack(np.concatenate([b_ioux + b_iouhL + b_iouhR, b_fx + b_fhL, b_fx + b_fhR]))
    bleaf = pack(np.concatenate([b_ioux + b_iouh, z, z]))
    b1 = pack(np.concatenate([b_ioux + b_iouhL, b_fx + b_fhL, z]))
    return b2, bleaf, b1


def prepare_in_maps(np_inputs):
    i = np_inputs
    inputs = np.asarray(i["inputs"], np.float32)
    w_f32 = _pack_w_big(
        i["W_ioux"], i["W_fx"], i["W_iouhL"], i["W_fhL"], i["W_iouhR"], i["W_fhR"]
    )
    w8_np = (w_f32 * WSCALE).astype(FP8)
    wb_np = (w_f32 * WSCALE).astype(BF16)
    b_args = [
        np.asarray(i[k], np.float32)
        for k in ("b_ioux", "b_iouh", "b_iouhL", "b_iouhR", "b_fx", "b_fhL", "b_fhR")
    ]
    b2, bleaf, b1 = pack_biases(*b_args)

    in_maps = []
    for c in range(NCORES):
        xc = inputs[c * BC : (c + 1) * BC]  # [BC, N, D]
        # [N, D, BC] -> kt-major [4, 128, N, BC]
        xt_c = xc.transpose(1, 2, 0).reshape(N, 4, 128, BC).transpose(1, 2, 0, 3)
        xt_c = np.ascontiguousarray(xt_c)
        in_maps.append(
            {
                "xt8": xt_c.astype(FP8),
                "xtb": np.ascontiguousarray(xt_c[:, :, :N_TAIL]).astype(BF16),
                "w8": w8_np,
                "wb": wb_np,
                "b2": b2,
                "bleaf": bleaf,
                "b1": b1,
            }
        )
    return in_maps


def kernel(
    inputs,
    W_ioux, b_ioux, W_iouh, b_iouh, W_iouhL, b_iouhL, W_iouhR, b_iouhR,
    W_fx, b_fx, W_fh, b_fh, W_fhL, b_fhL, W_fhR, b_fhR,
    left_idx, right_idx, num_children,
):
    el, er, en = _expected_tree()
    assert np.array_equal(np.asarray(left_idx), el), "unexpected tree structure"
    assert np.array_equal(np.asarray(right_idx), er), "unexpected tree structure"
    assert np.array_equal(np.asarray(num_children), en), "unexpected tree structure"

    in_maps = prepare_in_maps(
        dict(
            inputs=inputs,
            W_ioux=W_ioux, W_fx=W_fx, W_iouhL=W_iouhL, W_fhL=W_fhL,
            W_iouhR=W_iouhR, W_fhR=W_fhR,
            b_ioux=b_ioux, b_iouh=b_iouh, b_iouhL=b_iouhL, b_iouhR=b_iouhR,
            b_fx=b_fx, b_fhL=b_fhL, b_fhR=b_fhR,
        )
    )

    if "nc" not in _compiled:
        _compiled["nc"] = _build_bass()
    nc = _compiled["nc"]

    res = run_bass_kernel_spmd(
        nc, in_maps, core_ids=list(range(NCORES)), trace=bool(_compiled.get("trace"))
    )
    _compiled["last_res"] = res

    c_full = np.empty((B, D), np.float32)
    h_full = np.empty((B, D), np.float32)
    for c in range(NCORES):
        c_full[c * BC : (c + 1) * BC] = res.results[c]["c0t"].T
        h_full[c * BC : (c + 1) * BC] = res.results[c]["h0t"].T
    return c_full, h_full


# revision 52
# speedup vs baseline: 2.8672x; 1.4896x over previous
"""ConstituencyTreeLSTM Trainium2 kernel (fp8 DoubleRow edition).

Strategy:
  - Data-parallel over the B=256 batch across 8 NeuronCores (32 rows/core).
  - Complete-heap tree -> level-parallel phases:
      leaves (128..255) -> node 127 -> L6 (63..126) -> ... -> L1 -> root.
  - Feature-on-partitions layout; matmul PSUM outputs feed the next level
    without transposes. All SBUF tiles are kt-major: [128, kt, node, batch].
  - Big levels (leaves, 127, L6, L5, L4) run matmuls in fp8e4 with
    MatmulPerfMode.DoubleRow: 2 k-tiles contracted per instruction at
    0.5 cycles/row -> 4x bf16 PE throughput, half the instructions.
    Weights are scaled x64 host-side (keeps uniform(+-1/sqrt(512)) weights
    in the fp8 normal range); the 1/64 is folded into the PSUM-evacuating
    activation's scale. x and h are quantized to fp8 unscaled.
  - Small levels (L3..root, 15 nodes) run in bf16: their PE time is
    negligible and this restores most of the accuracy (sim: 1.3e-2 vs
    3.7e-2 all-fp8, threshold 2e-2).
  - h lives in SBUF per level, parity-split by parent (left-children tile /
    right-children tile) so DoubleRow rhs slices stay dense: the stride-2
    child gather becomes a contiguous slice indexed by parent.
  - c of big levels goes through DRAM (CL/CR, parity-split by parent);
    c of L4..L1 stays in SBUF.
  - Per-node-type biases (2-child / leaf / 1-child) folded host-side,
    applied inside the PSUM-evacuating activation (sigmoid/tanh).
  - Elementwise c/h stage on DVE (2x bf16) with one mul offloaded to Pool.
"""

import sys

sys.path.insert(0, "/opt/trn_rl_repo")

import numpy as np
import ml_dtypes

import concourse.bass as bass  # noqa: F401
import concourse.mybir as mybir
import concourse.tile as tile
from concourse import bacc
from concourse.bass_utils import run_bass_kernel_spmd

BF16 = ml_dtypes.bfloat16
FP8 = ml_dtypes.float8_e4m3
NCORES = 8
B, N, D = 256, 256, 512
BC = B // NCORES  # batch rows per core
NJ = 20  # output j-tiles: 12 iou + 4 fL + 4 fR
WSCALE = 64.0  # fp8 weight scale; inverse folded into activation scale

N_TAIL = 15  # nodes 0..14 (L3..root) run in bf16

_compiled = {}


def _used_kts(j, has_l=True, has_r=True):
    if j < 12:
        kts = list(range(0, 4)) + (list(range(4, 8)) if has_l else []) + (
            list(range(8, 12)) if has_r else []
        )
    elif j < 16:
        kts = list(range(0, 4)) + list(range(4, 8))
    else:
        kts = list(range(0, 4)) + list(range(8, 12))
    return kts


# packed weight-block index: only (kt, j) pairs with nonzero weight blocks.
# The x-blocks of j16..19 (fR's W_fx) duplicate j12..15's (fL's W_fx) and are
# not stored; widx() remaps them. Leaf-phase blocks (x-kts of the iou gates)
# come first so the leaf matmuls only wait on a small initial weight DMA.
W_LEAF = [(kt, j) for j in range(12) for kt in range(4)]
# rest ordered so node-127's blocks (hL of iou + all of fL) come first
W_REST = (
    [(kt, j) for j in range(12) for kt in range(4, 8)]
    + [(kt, j) for j in range(12, 16) for kt in range(8)]
    + [(kt, j) for j in range(12) for kt in range(8, 12)]
    + [(kt, j) for j in range(16, 20) for kt in range(8, 12)]
)
W_BLOCKS = W_LEAF + W_REST
NW_LEAF = len(W_LEAF)  # 48
W_IDX = {p: i for i, p in enumerate(W_BLOCKS)}
NW = len(W_BLOCKS)  # 192


def widx(kt, j):
    if j >= 16 and kt < 4:
        j = j - 4
    return W_IDX[(kt, j)]


def _build_bass(reps=1, skip_ew=False, skip_act=False):
    nc = bacc.Bacc("TRN2", target_bir_lowering=False, debug=False, num_devices=NCORES)

    f32 = mybir.dt.float32
    bf16 = mybir.dt.bfloat16
    f8 = mybir.dt.float8e4
    DR = mybir.MatmulPerfMode.DoubleRow

    # x/c DRAM tensors are stored kt-major [kt, 128, node, batch] so that
    # kt-major SBUF tiles DMA with <=3 free dims.
    xt8 = nc.dram_tensor("xt8", [4, 128, N, BC], f8, kind="ExternalInput")
    xtb = nc.dram_tensor("xtb", [4, 128, N_TAIL, BC], bf16, kind="ExternalInput")
    w8 = nc.dram_tensor("w8", [NW, 128, 128], f8, kind="ExternalInput")
    wb = nc.dram_tensor("wb", [NW, 128, 128], bf16, kind="ExternalInput")
    b2_d = nc.dram_tensor("b2", [128, NJ], f32, kind="ExternalInput")
    bleaf_d = nc.dram_tensor("bleaf", [128, NJ], f32, kind="ExternalInput")
    b1_d = nc.dram_tensor("b1", [128, NJ], f32, kind="ExternalInput")

    # children c keyed by parent index t: CL[t] = c(2t+1), CR[t] = c(2t+2)
    CL = nc.dram_tensor("CLbuf", [4, 128, 128, BC], bf16)
    CR = nc.dram_tensor("CRbuf", [4, 128, 128, BC], bf16)

    c0t = nc.dram_tensor("c0t", [D, BC], f32, kind="ExternalOutput")
    h0t = nc.dram_tensor("h0t", [D, BC], f32, kind="ExternalOutput")

    # kt-major views [partition, ktile, node, batch]
    xt8_r = xt8.ap().rearrange("kt p n b -> p kt n b")
    xtb_r = xtb.ap().rearrange("kt p n b -> p kt n b")
    CL_r = CL.ap().rearrange("kt p t b -> p kt t b")
    CR_r = CR.ap().rearrange("kt p t b -> p kt t b")
    c0t_r = c0t.ap().rearrange("(kt p) b -> p kt b", p=128)
    h0t_r = h0t.ap().rearrange("(kt p) b -> p kt b", p=128)

    with tile.TileContext(nc) as tc:
        import contextlib

        ctx = contextlib.ExitStack()
        with ctx:
            wpool = ctx.enter_context(tc.tile_pool(name="wpool", bufs=1))
            hpool = ctx.enter_context(tc.tile_pool(name="hpool", bufs=1))
            inpool = ctx.enter_context(tc.tile_pool(name="inpool", bufs=2))
            gpool = ctx.enter_context(tc.tile_pool(name="gpool", bufs=2))
            epool = ctx.enter_context(tc.tile_pool(name="epool", bufs=2))
            pspool = ctx.enter_context(tc.tile_pool(name="ps", bufs=8, space="PSUM"))

            w8_sb = wpool.tile([128, NW, 128], f8, name="w8sb")
            w8_r = w8.ap().rearrange("blk p c -> p blk c")
            # leaf-phase blocks first; everything else is queued as small
            # pieces interleaved between per-chunk loads (drained inside
            # process()) so no big transfer ever blocks a chunk load.
            nc.sync.dma_start(out=w8_sb[:, :NW_LEAF, :], in_=w8_r[:, :NW_LEAF, :])
            b2_sb = wpool.tile([128, NJ], f32, name="b2sb")
            bleaf_sb = wpool.tile([128, NJ], f32, name="bleafsb")
            b1_sb = wpool.tile([128, NJ], f32, name="b1sb")
            nc.sync.dma_start(out=b2_sb[:], in_=b2_d.ap()[:])
            nc.sync.dma_start(out=bleaf_sb[:], in_=bleaf_d.ap()[:])
            nc.sync.dma_start(out=b1_sb[:], in_=b1_d.ap()[:])
            wb_sb = wpool.tile([128, NW, 128], bf16, name="wbsb")
            xtb_t = wpool.tile([128, 4, N_TAIL, BC], bf16, name="xtb_t")
            nc.sync.dma_start(out=xtb_t[:], in_=xtb_r[:])
            # precomputed x-projections (x64 scale) for the tail nodes 0..14
            xp_sb = wpool.tile([128, NJ, N_TAIL, BC], bf16, name="xp_sb")
            wb_r = wb.ap().rearrange("blk p c -> p blk c")

            pending = []

            def _pend(sb, r, lo, hi):
                pending.append(
                    lambda: nc.sync.dma_start(out=sb[:, lo:hi, :], in_=r[:, lo:hi, :])
                )

            # order: n127's fp8 blocks, tailx's bf16 x blocks, L6's fp8 hR,
            # then the rest of the bf16 weights
            _pend(w8_sb, w8_r, 48, 96)
            _pend(w8_sb, w8_r, 96, 128)
            _pend(wb_sb, wb_r, 0, 48)
            _pend(wb_sb, wb_r, 96, 128)
            _pend(w8_sb, w8_r, 128, 176)
            _pend(w8_sb, w8_r, 176, 192)
            _pend(wb_sb, wb_r, 48, 96)
            _pend(wb_sb, wb_r, 128, 176)
            _pend(wb_sb, wb_r, 176, 192)

            def drain_pending(n=1):
                for _ in range(n):
                    if pending:
                        pending.pop(0)()

            def process(
                nodes,
                has_l,
                has_r,
                bias_sb,
                mode,  # 'fp8' | 'mix' (bf16 x, fp8 h) | 'bf16'
                child_h,  # (hL_tile, hR_tile, parent_base) or None
                out_h,  # (hL_cons, hR_cons, special) or None (root);
                # special: (tile, node) for a child outside the parent range
                child_c=None,  # (tile, base_node) -> children c from SBUF
                out_c=None,  # (tile, base_node) -> write c to SBUF, skip CL/CR
                chunk=None,  # emit only the chunk starting at this node
            ):
                """Compute (c, h) for `nodes` (a range), all at the same depth."""
                to_out = out_h is None
                starts = list(range(nodes.start, nodes.stop, 16))
                if chunk is not None:
                    starts = [chunk]
                for a in starts:
                    b_ = min(a + 16, nodes.stop)
                    k = b_ - a  # nodes in this chunk
                    dt_g = f32 if to_out else bf16

                    if mode == "fp8":
                        xt_t = inpool.tile([128, 4, k, BC], f8, name="xt_t")
                        nc.sync.dma_start(out=xt_t[:], in_=xt8_r[:, :, a:b_, :])
                    else:
                        xt_t = xtb_t[:, :, a : b_, :]
                    drain_pending()
                    if child_c is None and child_h is not None:
                        if has_l:
                            cl_t = inpool.tile([128, 4, k, BC], bf16, name="cl_t")
                            nc.scalar.dma_start(out=cl_t[:], in_=CL_r[:, :, a:b_, :])
                        if has_r:
                            cr_t = inpool.tile([128, 4, k, BC], bf16, name="cr_t")
                            nc.scalar.dma_start(out=cr_t[:], in_=CR_r[:, :, a:b_, :])
                    if child_c is not None:
                        cc_t, cc_base = child_c
                        cs0 = 2 * a + 1 - cc_base
                        if k == 1:
                            cl_t = cc_t[:, :, cs0 : cs0 + 1, :]
                            cr_t = cc_t[:, :, cs0 + 1 : cs0 + 2, :]
                        else:
                            cl_t = cc_t[:, :, cs0 : cs0 + 2 * k - 1 : 2, :]
                            cr_t = cc_t[:, :, cs0 + 1 : cs0 + 2 * k : 2, :]
                    if child_h is not None:
                        chL, chR, ch_base = child_h
                        s = a - ch_base

                        def hl_slice(q0, q1):
                            return chL[:, q0:q1, s : s + k, :]

                        def hr_slice(q0, q1):
                            return chR[:, q0:q1, s : s + k, :]

                    g_i = gpool.tile([128, 4, k, BC], dt_g, name="g_i", bufs=1)
                    g_o = gpool.tile([128, 4, k, BC], dt_g, name="g_o")
                    g_u = gpool.tile([128, 4, k, BC], dt_g, name="g_u", bufs=1)
                    if has_l:
                        g_fl = gpool.tile([128, 4, k, BC], dt_g, name="g_fl", bufs=1)
                    if has_r:
                        g_fr = gpool.tile([128, 4, k, BC], dt_g, name="g_fr", bufs=1)

                    # f-gates first so the fl*cl / fr*cr muls overlap the
                    # i/u/o matmuls; o last, right before h = o * tanh(c).
                    js = []
                    if has_r:
                        js += list(range(16, 20))
                    if has_l:
                        js += list(range(12, 16))
                    js += list(range(0, 4)) + list(range(8, 12)) + list(range(4, 8))

                    for j in js:
                        kts = _used_kts(j, has_l, has_r)
                        ps = pspool.tile([128, k, BC], f32, name="ps")
                        if mode in ("seed8", "seedb"):
                            # tail: x-part precomputed in xp_sb; h-matmuls
                            # (fp8 DoubleRow or bf16 singles) + DVE seed-add
                            hkts = [kt for kt in kts if kt >= 4]
                            if mode == "seed8":
                                prs = [
                                    (hkts[q], hkts[q + 1])
                                    for q in range(0, len(hkts), 2)
                                ]
                                for i, (kt0, kt1) in enumerate(prs):
                                    wi = widx(kt0, j)
                                    rhs = (
                                        hl_slice(kt0 - 4, kt0 - 2)
                                        if kt0 < 8
                                        else hr_slice(kt0 - 8, kt0 - 6)
                                    )
                                    nc.tensor.matmul(
                                        ps[:],
                                        w8_sb[:, wi : wi + 2, :],
                                        rhs,
                                        start=(i == 0),
                                        stop=(i == len(prs) - 1),
                                        perf_mode=DR,
                                    )
                            else:
                                for i, kt in enumerate(hkts):
                                    rhs = (
                                        chL[:, kt - 4, s : s + k, :]
                                        if kt < 8
                                        else chR[:, kt - 8, s : s + k, :]
                                    )
                                    nc.tensor.matmul(
                                        ps[:],
                                        wb_sb[:, widx(kt, j), :],
                                        rhs,
                                        start=(i == 0),
                                        stop=(i == len(hkts) - 1),
                                    )
                            nc.vector.tensor_add(
                                ps[:], ps[:], xp_sb[:, j, a:b_, :]
                            )
                        elif mode == "fp8":
                            pairs = [(kts[q], kts[q + 1]) for q in range(0, len(kts), 2)]
                            for i, (kt0, kt1) in enumerate(pairs):
                                wi = widx(kt0, j)
                                assert widx(kt1, j) == wi + 1
                                if kt0 < 4:
                                    rhs = xt_t[:, kt0 : kt0 + 2, :, :]
                                elif kt0 < 8:
                                    rhs = hl_slice(kt0 - 4, kt0 - 2)
                                else:
                                    rhs = hr_slice(kt0 - 8, kt0 - 6)
                                nc.tensor.matmul(
                                    ps[:],
                                    w8_sb[:, wi : wi + 2, :],
                                    rhs,
                                    start=(i == 0),
                                    stop=(i == len(pairs) - 1),
                                    perf_mode=DR,
                                )
                        elif mode == "mix":
                            # bf16 x singles + fp8 DoubleRow h pairs, one group
                            hkts = [kt for kt in kts if kt >= 4]
                            pairs = [
                                (hkts[q], hkts[q + 1]) for q in range(0, len(hkts), 2)
                            ]
                            for i, kt in enumerate(range(4)):
                                nc.tensor.matmul(
                                    ps[:],
                                    wb_sb[:, widx(kt, j), :],
                                    xt_t[:, kt, :, :],
                                    start=(i == 0),
                                    stop=False,
                                )
                            for i, (kt0, kt1) in enumerate(pairs):
                                wi = widx(kt0, j)
                                assert widx(kt1, j) == wi + 1
                                if kt0 < 8:
                                    rhs = hl_slice(kt0 - 4, kt0 - 2)
                                else:
                                    rhs = hr_slice(kt0 - 8, kt0 - 6)
                                nc.tensor.matmul(
                                    ps[:],
                                    w8_sb[:, wi : wi + 2, :],
                                    rhs,
                                    start=False,
                                    stop=(i == len(pairs) - 1),
                                    perf_mode=DR,
                                )
                        else:
                            for i, kt in enumerate(kts):
                                if kt < 4:
                                    rhs = xt_t[:, kt, :, :]
                                elif kt < 8:
                                    rhs = chL[:, kt - 4, s : s + k, :]
                                else:
                                    rhs = chR[:, kt - 8, s : s + k, :]
                                nc.tensor.matmul(
                                    ps[:],
                                    wb_sb[:, widx(kt, j), :],
                                    rhs,
                                    start=(i == 0),
                                    stop=(i == len(kts) - 1),
                                )
                        func = (
                            mybir.ActivationFunctionType.Tanh
                            if 8 <= j < 12
                            else mybir.ActivationFunctionType.Sigmoid
                        )
                        if j < 4:
                            dst = g_i[:, j, :, :]
                        elif j < 8:
                            dst = g_o[:, j - 4, :, :]
                        elif j < 12:
                            dst = g_u[:, j - 8, :, :]
                        elif j < 16:
                            dst = g_fl[:, j - 12, :, :]
                        else:
                            dst = g_fr[:, j - 16, :, :]
                        if not skip_act:
                            # both w8 and wb carry the x64 scale
                            nc.scalar.activation(
                                out=dst,
                                in_=ps[:],
                                func=func,
                                bias=bias_sb[:, j : j + 1],
                                scale=1.0 / WSCALE,
                            )

                    if skip_ew or skip_act:
                        continue
                    # c = i*u (+ fl*cl) (+ fr*cr);  h = o * tanh(c)
                    if out_c is not None:
                        oc_t, oc_base = out_c
                        c_t = oc_t[:, :, a - oc_base : b_ - oc_base, :]
                    else:
                        c_t = epool.tile([128, 4, k, BC], dt_g, name="c_t")
                    if has_r:
                        m3 = epool.tile([128, 4, k, BC], dt_g, name="m3", bufs=1)
                        eng = nc.gpsimd if k == 16 else nc.vector
                        eng.tensor_mul(m3[:], g_fr[:], cr_t[:])
                    if has_l:
                        m2 = epool.tile([128, 4, k, BC], dt_g, name="m2", bufs=1)
                        nc.vector.tensor_mul(m2[:], g_fl[:], cl_t[:])
                    nc.vector.tensor_mul(c_t[:], g_i[:], g_u[:])
                    if has_l:
                        nc.vector.tensor_add(c_t[:], c_t[:], m2[:])
                    if has_r:
                        nc.vector.tensor_add(c_t[:], c_t[:], m3[:])
                    tc_t = epool.tile([128, 4, k, BC], dt_g, name="tc_t")
                    nc.scalar.activation(
                        out=tc_t[:], in_=c_t[:], func=mybir.ActivationFunctionType.Tanh
                    )

                    if to_out:
                        h_t = epool.tile([128, 4, k, BC], dt_g, name="h_t")
                        nc.vector.tensor_mul(h_t[:], g_o[:], tc_t[:])
                        nc.scalar.dma_start(out=c0t_r[:], in_=c_t[:, :, 0, :])
                        nc.scalar.dma_start(out=h0t_r[:], in_=h_t[:, :, 0, :])
                        continue

                    # h: odd nodes -> left-child slot of parent, evens -> right
                    hL_cons, hR_cons, special = out_h
                    odd0 = 0 if a % 2 == 1 else 1
                    even0 = 1 - odd0
                    odds = list(range(a + odd0, b_, 2))
                    evens = list(range(a + even0, b_, 2))
                    if special is not None:
                        sp_tile, sp_node = special
                        if sp_node in odds:
                            odds.remove(sp_node)
                            io = sp_node - a
                            nc.vector.tensor_mul(
                                sp_tile[:, :, 0:1, :],
                                g_o[:, :, io : io + 1, :],
                                tc_t[:, :, io : io + 1, :],
                            )
                    if odds:
                        tile_, base = hL_cons
                        lo = (odds[0] - 1) // 2 - base
                        nc.vector.tensor_mul(
                            tile_[:, :, lo : lo + len(odds), :],
                            g_o[:, :, odd0 : odd0 + 2 * len(odds) - 1 : 2, :],
                            tc_t[:, :, odd0 : odd0 + 2 * len(odds) - 1 : 2, :],
                        )
                    if evens:
                        tile_, base = hR_cons
                        ro = evens[0] // 2 - 1 - base
                        nc.vector.tensor_mul(
                            tile_[:, :, ro : ro + len(evens), :],
                            g_o[:, :, even0 : even0 + 2 * len(evens) - 1 : 2, :],
                            tc_t[:, :, even0 : even0 + 2 * len(evens) - 1 : 2, :],
                        )

                    if out_c is not None:
                        continue  # c already written to its SBUF level tile
                    # c of node t -> CL[(t-1)//2] if t odd else CR[t//2 - 1]
                    # (per-kt DMAs: the stride-2 node slice + kt dim exceeds
                    # the 3-dim DMA AP limit otherwise)
                    all_odds = list(range(a + odd0, b_, 2))
                    for kt in range(4):
                        if all_odds:
                            lo = (all_odds[0] - 1) // 2
                            nc.scalar.dma_start(
                                out=CL_r[:, kt, lo : lo + len(all_odds), :],
                                in_=c_t[:, kt, odd0::2, :],
                            )
                        if evens:
                            ro = evens[0] // 2 - 1
                            nc.scalar.dma_start(
                                out=CR_r[:, kt, ro : ro + len(evens), :],
                                in_=c_t[:, kt, even0::2, :],
                            )

            # c stays in SBUF for the small tail levels (outputs of L4..L1);
            # their parent phases then skip the CL/CR DRAM roundtrip entirely.
            C_SBUF_LVLS = (4, 3, 2, 1)
            # per-phase precision: big levels full fp8; L3/L2 keep x in bf16
            # (the accuracy-sensitive path) with fp8 DoubleRow h; L1+root bf16
            PH_MODE = {
                7: "fp8", 6: "fp8", 5: "fp8", 4: "fp8",
                3: "seed8", 2: "seed8", 1: "seedb", 0: "seedb",
            }

            for _rep in range(reps):
                # per-consumer-level h tiles (SBUF-resident), parity-split.
                # hL[lvl]/hR[lvl] feed the phase processing level `lvl`:
                # slot (t - base(lvl)) holds h of left/right child of node t.
                hL, hR = {}, {}
                for lvl in range(7):
                    n_lvl = 2**lvl
                    dt_h = f8 if PH_MODE[lvl] in ("fp8", "mix", "seed8") else bf16
                    hL[lvl] = (
                        hpool.tile([128, 4, n_lvl, BC], dt_h, name=f"hL_{lvl}"),
                        n_lvl - 1,
                    )
                    hR[lvl] = (
                        hpool.tile([128, 4, n_lvl, BC], dt_h, name=f"hR_{lvl}"),
                        n_lvl - 1,
                    )
                hX127 = hpool.tile([128, 4, 1, BC], f8, name="hX127")  # h of node 255
                lvl_c = {}
                for lvl in C_SBUF_LVLS:
                    tl = hpool.tile([128, 4, 2**lvl, BC], bf16, name=f"c_{lvl}")
                    lvl_c[lvl] = (tl, 2**lvl - 1)

                if skip_ew or skip_act:
                    # ablation modes never write h/c tiles; zero them so the
                    # tile framework sees a writer for every read
                    for t, _ in list(hL.values()) + list(hR.values()):
                        nc.vector.memset(t[:], 0.0)
                    nc.vector.memset(hX127[:], 0.0)
                    for t, _ in lvl_c.values():
                        nc.vector.memset(t[:], 0.0)

                def emit_leaf(chunk):
                    # leaves: nodes 128..255 (no children). Node 255 -> hX127.
                    process(
                        range(128, 256),
                        False,
                        False,
                        bleaf_sb,
                        "fp8",
                        None,
                        (hL[6], hR[6], (hX127, 255)),
                        chunk=chunk,
                    )

                def emit_tailx(g):
                    # batched x-projection for tail nodes 0..14, one gate
                    # (4 j-tiles) per call; runs in the leaf window.
                    drain_pending()
                    for jj in range(4):
                        j = 4 * g + jj
                        ps = pspool.tile([128, N_TAIL, BC], f32, name="ps")
                        for i, kt in enumerate(range(4)):
                            nc.tensor.matmul(
                                ps[:],
                                wb_sb[:, widx(kt, j), :],
                                xtb_t[:, kt, :, :],
                                start=(i == 0),
                                stop=(i == 3),
                            )
                        nc.vector.tensor_copy(xp_sb[:, j, :, :], ps[:])

                def emit_127():
                    # node 127: left child only (255 in hX127); h -> hL6 slot 0
                    process(
                        range(127, 128),
                        True,
                        False,
                        b1_sb,
                        "fp8",
                        (hX127, None, 127),
                        (hL[6], hR[6], None),
                    )

                def emit_lvl(lvl, chunk=None):
                    if lvl == 0:
                        process(
                            range(0, 1),
                            True,
                            True,
                            b2_sb,
                            PH_MODE[0],
                            (hL[0][0], hR[0][0], 0),
                            None,
                            child_c=lvl_c.get(1),
                        )
                        return
                    process(
                        range(2**lvl - 1, 2 ** (lvl + 1) - 1),
                        True,
                        True,
                        b2_sb,
                        PH_MODE[lvl],
                        (hL[lvl][0], hR[lvl][0], hL[lvl][1]),
                        (hL[lvl - 1], hR[lvl - 1], None),
                        child_c=lvl_c.get(lvl + 1),
                        out_c=lvl_c.get(lvl),
                        chunk=chunk,
                    )

                # Interleaved emission: PE sits idle during the ACT-bound
                # leaf phase, so L6 chunks are emitted as soon as the leaf
                # chunks they read from (children 2t+1..2t+2) are emitted.
                emit_leaf(240)  # node 255 first: n127 and all of L6 need it
                emit_leaf(128)
                emit_tailx(0)
                emit_leaf(144)
                emit_tailx(1)
                emit_127()
                emit_lvl(6, 63)  # children 127..158
                emit_leaf(160)
                emit_tailx(2)
                emit_leaf(176)
                emit_lvl(6, 79)  # children 159..190
                emit_leaf(192)
                emit_tailx(3)
                emit_leaf(208)
                emit_lvl(6, 95)  # children 191..222
                emit_leaf(224)
                emit_tailx(4)
                emit_lvl(6, 111)  # children 223..254 (224.. emitted; 240 early)
                emit_lvl(5, 31)
                emit_lvl(5, 47)
                for lvl in range(4, -1, -1):
                    emit_lvl(lvl)

    nc.compile()
    return nc


def _expected_tree():
    left = np.array([2 * i + 1 if 2 * i + 1 < N else 0 for i in range(N)], np.int32)
    right = np.array([2 * i + 2 if 2 * i + 2 < N else 0 for i in range(N)], np.int32)
    nch = np.array(
        [int(2 * i + 1 < N) + int(2 * i + 2 < N) for i in range(N)], np.int32
    )
    return left, right, nch


def _pack_w_big(W_ioux, W_fx, W_iouhL, W_fhL, W_iouhR, W_fhR):
    w_big = np.zeros((1536, 2560), np.float32)
    w_big[0:512, 0:1536] = np.asarray(W_ioux, np.float32).T
    w_big[0:512, 1536:2048] = np.asarray(W_fx, np.float32).T
    w_big[0:512, 2048:2560] = np.asarray(W_fx, np.float32).T
    w_big[512:1024, 0:1536] = np.asarray(W_iouhL, np.float32).T
    w_big[512:1024, 1536:2048] = np.asarray(W_fhL, np.float32).T
    w_big[1024:1536, 0:1536] = np.asarray(W_iouhR, np.float32).T
    w_big[1024:1536, 2048:2560] = np.asarray(W_fhR, np.float32).T
    w_np = np.empty((NW, 128, 128), np.float32)
    for i, (kt, j) in enumerate(W_BLOCKS):
        w_np[i] = w_big[kt * 128 : (kt + 1) * 128, j * 128 : (j + 1) * 128]
    return np.ascontiguousarray(w_np)


def pack_biases(b_ioux, b_iouh, b_iouhL, b_iouhR, b_fx, b_fhL, b_fhR):
    def pack(vec):
        return np.ascontiguousarray(np.asarray(vec, np.float32).reshape(NJ, 128).T)

    z = np.zeros(512, np.float32)
    b2 = pack(np.concatenate([b_ioux + b_iouhL + b_iouhR, b_fx + b_fhL, b_fx + b_fhR]))
    bleaf = pack(np.concatenate([b_ioux + b_iouh, z, z]))
    b1 = pack(np.concatenate([b_ioux + b_iouhL, b_fx + b_fhL, z]))
    return b2, bleaf, b1


def prepare_in_maps(np_inputs):
    i = np_inputs
    inputs = np.asarray(i["inputs"], np.float32)
    w_f32 = _pack_w_big(
        i["W_ioux"], i["W_fx"], i["W_iouhL"], i["W_fhL"], i["W_iouhR"], i["W_fhR"]
    )
    w8_np = (w_f32 * WSCALE).astype(FP8)
    wb_np = (w_f32 * WSCALE).astype(BF16)
    b_args = [
        np.asarray(i[k], np.float32)
        for k in ("b_ioux", "b_iouh", "b_iouhL", "b_iouhR", "b_fx", "b_fhL", "b_fhR")
    ]
    b2, bleaf, b1 = pack_biases(*b_args)

    in_maps = []
    for c in range(NCORES):
        xc = inputs[c * BC : (c + 1) * BC]  # [BC, N, D]
        # [N, D, BC] -> kt-major [4, 128, N, BC]
        xt_c = xc.transpose(1, 2, 0).reshape(N, 4, 128, BC).transpose(1, 2, 0, 3)
        xt_c = np.ascontiguousarray(xt_c)
        in_maps.append(
            {
                "xt8": xt_c.astype(FP8),
                "xtb": np.ascontiguousarray(xt_c[:, :, :N_TAIL]).astype(BF16),
                "w8": w8_np,
                "wb": wb_np,
                "b2": b2,
                "bleaf": bleaf,
                "b1": b1,
            }
        )
    return in_maps


def kernel(
    inputs,
    W_ioux, b_ioux, W_iouh, b_iouh, W_iouhL, b_iouhL, W_iouhR, b_iouhR,
    W_fx, b_fx, W_fh, b_fh, W_fhL, b_fhL, W_fhR, b_fhR,
    left_idx, right_idx, num_children,
):
    el, er, en = _expected_tree()
    assert np.array_equal(np.asarray(left_idx), el), "unexpected tree structure"
    assert np.array_equal(np.asarray(right_idx), er), "unexpected tree structure"
    assert np.array_equal(np.asarray(num_children), en), "unexpected tree structure"

    in_maps = prepare_in_maps(
        dict(
            inputs=inputs,
            W_ioux=W_ioux, W_fx=W_fx, W_iouhL=W_iouhL, W_fhL=W_fhL,
            W_iouhR=W_iouhR, W_fhR=W_fhR,
            b_ioux=b_ioux, b_iouh=b_iouh, b_iouhL=b_iouhL, b_iouhR=b_iouhR,
            b_fx=b_fx, b_fhL=b_fhL, b_fhR=b_fhR,
        )
    )

    if "nc" not in _compiled:
        _compiled["nc"] = _build_bass()
    nc = _compiled["nc"]

    res = run_bass_kernel_spmd(
        nc, in_maps, core_ids=list(range(NCORES)), trace=bool(_compiled.get("trace"))
    )
    _compiled["last_res"] = res

    c_full = np.empty((B, D), np.float32)
    h_full = np.empty((B, D), np.float32)
    for c in range(NCORES):
        c_full[c * BC : (c + 1) * BC] = res.results[c]["c0t"].T
        h_full[c * BC : (c + 1) * BC] = res.results[c]["h0t"].T
    return c_full, h_full
